# revision 1
# baseline (speedup 1.0000x reference)
"""BidirectionalAttention Trainium2 Bass kernel — 8-core SPMD.

Decomposition (verified against the oracle in fp32, rel-err 2.9e-7):
  q path : 1x1 conv (matmul) -> grouped conv1d k=3 -> conv1d k=3
  attn   : E = exp(q^T k) without max-subtraction (attn absmax ~6.5);
           both softmaxes share one exp:
             attn_f + attn_b = E * (1/S0[n,m] + 1/S1[b,m])
             S0 = sum_b E  (batch softmax denom, axis=0)
             S1 = sum_n E  (row softmax denom, axis=1)
  fusion = value @ (attn_f+attn_b)^T scaled by gamma*mean(x_b), + x
  ConvTranspose2d(k=4,s=2,p=1) via the 4-subkernel parity decomposition.

Sharding: sequence-parallel over attention rows n (HW=4096 -> 512 rows/core =
8 image rows).  Per core E is stored transposed [m, (b, n_loc)] in SBUF
(bf16, 32 tiles of [128, 4, 512], one exp per m-tile):
  - S0 (sum over batch) is local elementwise over the 4 batch slices
  - S1 (sum over n) is a free-dim sum (DVE tensor_scalar accum_out), then
    two small AllReduces (split in half so the first half of the fusion
    matmuls can start while the second half of QK/exp still runs).
K and V^T shards are exchanged with small AllGathers (K per batch, early).
The ConvTranspose needs fusion rows h0-1..h0+8; instead of a halo exchange
each core emits an 18-row output slab with *partial* sums on the 2-row
boundaries and the host stitches slabs by adding the overlaps
(transposed-conv contributions are additive), keeping the device program
rank-independent.
"""

import numpy as np

B = 4
C = 256
H = 64
Wd = 64
HW = H * Wd            # 4096
CR = 32                # C // 8
NCORES = 8
NL = HW // NCORES      # 512 owned attention rows (n) per core
HL = H // NCORES       # 8 owned image rows per core
MT = HW // 128         # 32 m-tiles of 128
XW = NL + 4            # x slab width (n halo +-2 for the two k=3 convs)
Q2W = NL + 2           # q2 width (halo +-1 for conv2)
ROWW = 68              # fusion_pad row width: [0,1]=zero, 2..65 data, [66,67]=zero
OUTROWS = 2 * HL + 2   # 18 output rows per core (2-row overlaps, host-stitched)

_CACHE = {}


# ---------------------------------------------------------------------------
# device module
# ---------------------------------------------------------------------------
def build_module():
    from contextlib import ExitStack

    import concourse.bass as bass
    import concourse.mybir as mybir
    from concourse import bacc
    from concourse.tile import TileContext

    f32 = mybir.dt.float32
    bf16 = mybir.dt.bfloat16
    AF = mybir.ActivationFunctionType
    OP = mybir.AluOpType
    AX = mybir.AxisListType

    nc = bacc.Bacc(num_devices=NCORES)
    RG = [list(range(NCORES))]

    # ---- parameters (per-core) -------------------------------------------
    xs_p = nc.declare_dram_parameter("xs", [B, C, XW], bf16, isOutput=False)
    wqT_p = nc.declare_dram_parameter("wqT", [C, C], bf16, isOutput=False)
    wvT_p = nc.declare_dram_parameter("wvT", [C, C], bf16, isOutput=False)
    w1_p = nc.declare_dram_parameter("w1", [3, C, CR], bf16, isOutput=False)
    w2_p = nc.declare_dram_parameter("w2", [3, CR, 2 * CR], bf16, isOutput=False)
    wco_p = nc.declare_dram_parameter("wco", [4, 4, C, C // 2], bf16, isOutput=False)
    bq_p = nc.declare_dram_parameter("bq", [C, 1], f32, isOutput=False)
    b1_p = nc.declare_dram_parameter("b1", [CR, 1], f32, isOutput=False)
    b2_p = nc.declare_dram_parameter("b2p", [2 * CR, 1], f32, isOutput=False)
    bco_p = nc.declare_dram_parameter("bco", [C // 2, 1], f32, isOutput=False)
    bvb_p = nc.declare_dram_parameter("bvb", [128, C], bf16, isOutput=False)
    mask_p = nc.declare_dram_parameter("mask", [128, XW], bf16, isOutput=False)
    gamma_p = nc.declare_dram_parameter("gammas", [1, 1], f32, isOutput=False)
    out_p = nc.declare_dram_parameter(
        "out", [B, C // 2, OUTROWS, 2 * Wd], f32, isOutput=True
    )

    with TileContext(nc) as tc, ExitStack() as ctx:
        # ---- long-lived pools -------------------------------------------
        const = ctx.enter_context(tc.tile_pool(name="const", bufs=1))
        xpool = ctx.enter_context(tc.tile_pool(name="xp", bufs=1))
        qkv = ctx.enter_context(tc.tile_pool(name="qkv", bufs=1))
        epool = ctx.enter_context(tc.tile_pool(name="E", bufs=1))
        fpool = ctx.enter_context(tc.tile_pool(name="fp", bufs=1))
        dram = ctx.enter_context(tc.tile_pool(name="dram", bufs=1, space="DRAM"))

        # ---- DRAM bounce buffers ----------------------------------------
        k_in = [dram.tile([CR, NL], bf16, tag=f"k_in{b}", name=f"k_in{b}") for b in range(B)]
        k_out = [
            dram.tile([NCORES, CR, NL], bf16, tag=f"k_out{b}", name=f"k_out{b}")
            for b in range(B)
        ]
        v_in = dram.tile([B, NL, C], bf16, tag="v_in", name="v_in")
        v_out = dram.tile([NCORES, B, NL, C], bf16, tag="v_out", name="v_out")
        ar1_in = dram.tile([128, 64], f32, tag="ar1_in", name="ar1_in")
        ar1_out = dram.tile([128, 64], f32, tag="ar1_out", name="ar1_out")
        ar2_in = dram.tile([128, 72], f32, tag="ar2_in", name="ar2_in")
        ar2_out = dram.tile([128, 72], f32, tag="ar2_out", name="ar2_out")
        g_dram = dram.tile([1, B], f32, tag="g_dram", name="g_dram")

        # ---- constants into SBUF ----------------------------------------
        wq_sb = [const.tile([128, C], bf16, tag=f"wq{k}", name=f"wq{k}") for k in range(2)]
        wv_sb = [const.tile([128, C], bf16, tag=f"wv{k}", name=f"wv{k}") for k in range(2)]
        for k in range(2):
            nc.sync.dma_start(out=wq_sb[k], in_=wqT_p[k * 128 : (k + 1) * 128, :])
            nc.sync.dma_start(out=wv_sb[k], in_=wvT_p[k * 128 : (k + 1) * 128, :])
        w1_sb = [
            [const.tile([128, CR], bf16, tag=f"w1_{t}_{k}", name=f"w1_{t}_{k}") for k in range(2)]
            for t in range(3)
        ]
        for t in range(3):
            for k in range(2):
                nc.sync.dma_start(
                    out=w1_sb[t][k], in_=w1_p[t, k * 128 : (k + 1) * 128, :]
                )
        w2_sb = [const.tile([CR, 2 * CR], bf16, tag=f"w2_{t}", name=f"w2_{t}") for t in range(3)]
        for t in range(3):
            nc.sync.dma_start(out=w2_sb[t], in_=w2_p[t])
        bq_sb = [const.tile([128, 1], f32, tag=f"bq{k}", name=f"bq{k}") for k in range(2)]
        for k in range(2):
            nc.sync.dma_start(out=bq_sb[k], in_=bq_p[k * 128 : (k + 1) * 128, :])
        b1_sb = const.tile([CR, 1], f32, tag="b1", name="b1")
        nc.sync.dma_start(out=b1_sb, in_=b1_p[:, :])
        b2_sb = const.tile([2 * CR, 1], f32, tag="b2", name="b2")
        nc.sync.dma_start(out=b2_sb, in_=b2_p[:, :])
        bco_sb = const.tile([128, 1], f32, tag="bco", name="bco")
        nc.sync.dma_start(out=bco_sb, in_=bco_p[:, :])
        bvb_sb = const.tile([128, C], bf16, tag="bvb", name="bvb")
        nc.sync.dma_start(out=bvb_sb, in_=bvb_p[:, :])
        mask_sb = const.tile([128, XW], bf16, tag="mask", name="mask")
        nc.sync.dma_start(out=mask_sb, in_=mask_p[:, :])
        gm_sb = const.tile([1, 1], f32, tag="gm", name="gm")
        nc.sync.dma_start(out=gm_sb, in_=gamma_p[:, :])
        wco_sb = [
            [
                [const.tile([128, 128], bf16, tag=f"wco{ky}_{kx}_{k}", name=f"wco{ky}_{kx}_{k}") for k in range(2)]
                for kx in range(4)
            ]
            for ky in range(4)
        ]
        for ky in range(4):
            for kx in range(4):
                for k in range(2):
                    nc.sync.dma_start(
                        out=wco_sb[ky][kx][k],
                        in_=wco_p[ky, kx, k * 128 : (k + 1) * 128, :],
                    )

        # ---- x load (already bf16 + zero-padded halo on host) -----------
        x_sb = [
            [xpool.tile([128, XW], bf16, tag=f"x{b}_{k}", name=f"x{b}_{k}") for k in range(2)]
            for b in range(B)
        ]
        for b in range(B):
            for k in range(2):
                nc.sync.dma_start(
                    out=x_sb[b][k], in_=xs_p[b, k * 128 : (k + 1) * 128, :]
                )

        tc.strict_bb_all_engine_barrier()

        # s1p: S1 partials at col mt*4+b (cols 0..127), x partial sums at
        # cols 128 + b*2 + k.  AllReduced in two halves.
        s1p_sb = qkv.tile([128, 136], f32, tag="s1p", name="s1p")
        for b in range(B):
            for k in range(2):
                cc = 128 + b * 2 + k
                nc.vector.tensor_reduce(
                    out=s1p_sb[:, cc : cc + 1],
                    in_=x_sb[b][k][:, 2 : 2 + NL],
                    axis=AX.X,
                    op=OP.add,
                )

        Q_all = qkv.tile([128, NL], bf16, tag="Q", name="Q")
        K_all = qkv.tile([128, HW], bf16, tag="K", name="K")
        r1_sb = qkv.tile([128, 128], f32, tag="r1", name="r1")  # 1/S1, col mt*4+b
        g_bcast = qkv.tile([128, B], f32, tag="gbc", name="gbc")

        # =================================================================
        # phase A: q path (per batch); phase B: value path
        # =================================================================
        with (
            tc.tile_pool(name="qtmp", bufs=2) as qtmp,
            tc.tile_pool(name="qps", bufs=2, space="PSUM") as qps,
            tc.tile_pool(name="q2ps", bufs=1, space="PSUM") as q2ps,
            tc.tile_pool(name="q3ps", bufs=1, space="PSUM") as q3ps,
            tc.tile_pool(name="vps", bufs=1, space="PSUM") as vps,
        ):
            for b in range(B):
                # ---- q1 = wq @ x + bq, then edge-mask -------------------
                q1_sb = []
                for mtile in range(2):
                    ps = qps.tile([128, XW], f32, tag="q1ps", name="q1ps")
                    for k in range(2):
                        for lo, hi in ((0, 512), (512, XW)):
                            nc.tensor.matmul(
                                ps[:, lo:hi],
                                wq_sb[k][:, mtile * 128 : (mtile + 1) * 128],
                                x_sb[b][k][:, lo:hi],
                                start=(k == 0),
                                stop=(k == 1),
                            )
                    q1 = qtmp.tile([128, XW], bf16, tag=f"q1_{mtile}", name=f"q1_{mtile}")
                    nc.scalar.activation(
                        out=q1, in_=ps, func=AF.Identity, bias=bq_sb[mtile]
                    )
                    nc.vector.tensor_mul(q1, q1, mask_sb)
                    q1_sb.append(q1)

                # ---- q2 = groupedconv(q1) + b1, then edge-mask ----------
                ps2 = q2ps.tile([CR, Q2W], f32, tag="q2ps", name="q2ps")
                for t in range(3):
                    for k in range(2):
                        st = t == 0 and k == 0
                        sp = t == 2 and k == 1
                        for lo, hi in ((0, 512), (512, Q2W)):
                            nc.tensor.matmul(
                                ps2[:, lo:hi],
                                w1_sb[t][k],
                                q1_sb[k][:, lo + t : hi + t],
                                start=st,
                                stop=sp,
                            )
                q2 = qtmp.tile([CR, Q2W], bf16, tag="q2", name="q2")
                nc.scalar.activation(out=q2, in_=ps2, func=AF.Identity, bias=b1_sb)
                nc.vector.tensor_mul(q2, q2, mask_sb[:CR, 1 : 1 + Q2W])

                # ---- q3 = conv(q2) + b2 (rows 0..31 query, 32..63 key) --
                ps3 = q3ps.tile([2 * CR, NL], f32, tag="q3ps", name="q3ps")
                for t in range(3):
                    nc.tensor.matmul(
                        ps3,
                        w2_sb[t],
                        q2[:, t : t + NL],
                        start=(t == 0),
                        stop=(t == 2),
                    )
                q3 = qtmp.tile([2 * CR, NL], bf16, tag="q3", name="q3")
                nc.scalar.activation(out=q3, in_=ps3, func=AF.Identity, bias=b2_sb)
                nc.sync.dma_start(
                    out=Q_all[32 * b : 32 * b + 32, :], in_=q3[0:CR, :]
                )
                nc.sync.dma_start(out=k_in[b][:, :], in_=q3[CR : 2 * CR, :])
                # gather this batch's key shard early (overlaps the rest)
                nc.gpsimd.collective_compute(
                    "AllGather",
                    OP.bypass,
                    replica_groups=RG,
                    ins=[k_in[b][:, :]],
                    outs=[k_out[b][:, :, :]],
                )
                nc.sync.dma_start(
                    out=K_all[32 * b : 32 * b + 32, :].rearrange(
                        "c (g m) -> c g m", g=NCORES
                    ),
                    in_=k_out[b][:, :, :].rearrange("g c m -> c g m"),
                )

                # ---- value^T shard: [m, c] = x^T @ wv^T + bv ------------
                for ms in range(4):
                    psv = vps.tile([128, C], f32, tag="vps", name="vps")
                    for k in range(2):
                        nc.tensor.matmul(
                            psv,
                            x_sb[b][k][:, 2 + ms * 128 : 2 + (ms + 1) * 128],
                            wv_sb[k],
                            start=(k == 0),
                            stop=(k == 1),
                        )
                    vt = qtmp.tile([128, C], bf16, tag="vt", name="vt")
                    nc.vector.tensor_add(vt, psv, bvb_sb)
                    nc.sync.dma_start(
                        out=v_in[b, ms * 128 : (ms + 1) * 128, :], in_=vt
                    )

        nc.gpsimd.collective_compute(
            "AllGather",
            OP.bypass,
            replica_groups=RG,
            ins=[v_in[:, :, :]],
            outs=[v_out[:, :, :, :]],
        )

        # =================================================================
        # phase C: E = exp(K^T Q), one [128, 4x512] tile per m-tile.
        # S1 partials via DVE tensor_scalar accum; AllReduce in two halves.
        # =================================================================
        e_sb = [None] * MT
        with (
            tc.tile_pool(name="qk", bufs=2, space="PSUM") as qk,
            tc.tile_pool(name="sc", bufs=2) as sc,
        ):
            for mt in range(MT):
                ps4 = qk.tile([128, B, NL], f32, tag="e4ps", name="e4ps")
                for b in range(B):
                    nc.tensor.matmul(
                        ps4[:, b, :],
                        K_all[32 * b : 32 * b + 32, mt * 128 : (mt + 1) * 128],
                        Q_all[32 * b : 32 * b + 32, :],
                        start=True,
                        stop=True,
                        tile_position=(32 * b, 0),
                    )
                e4 = epool.tile([128, B, NL], bf16, tag=f"e{mt}", name=f"e{mt}")
                nc.scalar.activation(out=e4, in_=ps4, func=AF.Exp)
                e_sb[mt] = e4
                # S1 partials: free-dim accumulate on DVE (4x mode copy)
                for b in range(B):
                    scr = sc.tile([128, NL], bf16, tag="scr", name="scr")
                    col = mt * 4 + b
                    nc.vector.tensor_scalar(
                        out=scr,
                        in0=e4[:, b, :],
                        scalar1=1.0,
                        scalar2=None,
                        op0=OP.mult,
                        op1=OP.add,
                        accum_out=s1p_sb[:, col : col + 1],
                    )

                if mt == MT // 2 - 1:
                    nc.sync.dma_start(out=ar1_in[:, :], in_=s1p_sb[:, 0:64])
                    nc.gpsimd.collective_compute(
                        "AllReduce", OP.add, replica_groups=RG,
                        ins=[ar1_in[:, :]], outs=[ar1_out[:, :]],
                    )
                    a1o = qkv.tile([128, 64], f32, tag="a1o", name="a1o")
                    nc.sync.dma_start(out=a1o, in_=ar1_out[:, :])
                    nc.vector.reciprocal_approx_fast(out=r1_sb[:, 0:64], in_=a1o)

            # second AR half: S1 cols 64..128 plus the x sums
            nc.sync.dma_start(out=ar2_in[:, 0:64], in_=s1p_sb[:, 64:128])
            nc.sync.dma_start(out=ar2_in[:, 64:72], in_=s1p_sb[:, 128:136])
            nc.gpsimd.collective_compute(
                "AllReduce", OP.add, replica_groups=RG,
                ins=[ar2_in[:, :]], outs=[ar2_out[:, :]],
            )
            a2o = qkv.tile([128, 72], f32, tag="a2o", name="a2o")
            nc.sync.dma_start(out=a2o, in_=ar2_out[:, :])
            nc.vector.reciprocal_approx_fast(out=r1_sb[:, 64:128], in_=a2o[:, 0:64])

            # g_bcast[p, b] = gamma * mean(x[b]): partition-reduce on gpsimd,
            # tiny math on partition 0, broadcast via 0-stride DMA from DRAM.
            xps = sc.tile([1, 8], f32, tag="xps", name="xps")
            nc.gpsimd.tensor_reduce(
                out=xps, in_=a2o[:, 64:72], axis=AX.C, op=OP.add
            )
            xv = xps.rearrange("p (b k) -> p b k", b=B)
            g0 = sc.tile([1, B], f32, tag="g0", name="g0")
            nc.vector.tensor_add(g0, xv[:, :, 0], xv[:, :, 1])
            nc.vector.tensor_scalar(
                out=g0,
                in0=g0,
                scalar1=gm_sb,
                scalar2=float(1.0 / (C * HW)),
                op0=OP.mult,
                op1=OP.mult,
            )
            nc.sync.dma_start(out=g_dram[:, :], in_=g0)
            nc.sync.dma_start(
                out=g_bcast,
                in_=bass.AP(
                    tensor=g_dram.tensor,
                    offset=g_dram.offset,
                    ap=[[0, 128], [1, B]],
                ),
            )

        # =================================================================
        # phase D: R = 1/S0; A = E*(R + r1b) in place; fusion matmuls
        # =================================================================
        fp_sb = [
            [fpool.tile([128, 10, ROWW], bf16, tag=f"fpad{b}_{ch}", name=f"fpad{b}_{ch}") for ch in range(2)]
            for b in range(B)
        ]
        for b in range(B):
            for ch in range(2):
                nc.gpsimd.memset(fp_sb[b][ch], 0.0)

        with (
            tc.tile_pool(name="fus", bufs=1, space="PSUM") as fus,
            tc.tile_pool(name="vtp", bufs=4) as vtp,
            tc.tile_pool(name="sp2", bufs=2) as sp2,
        ):
            fusion_ps = [
                [fus.tile([128, NL], f32, tag=f"f{b}_{ch}", name=f"f{b}_{ch}") for ch in range(2)]
                for b in range(B)
            ]
            for mt in range(MT):
                e4 = e_sb[mt]
                # S0 = sum_b E on gpsimd (idle engine), recip+cast on DVE
                s01 = sp2.tile([128, NL], bf16, tag="s01", name="s01")
                nc.gpsimd.tensor_add(s01, e4[:, 0, :], e4[:, 1, :])
                s23 = sp2.tile([128, NL], bf16, tag="s23", name="s23")
                nc.gpsimd.tensor_add(s23, e4[:, 2, :], e4[:, 3, :])
                s0f = sp2.tile([128, NL], f32, tag="s0f", name="s0f")
                nc.gpsimd.tensor_add(s0f, s01, s23)
                rf = sp2.tile([128, NL], f32, tag="rf", name="rf")
                nc.vector.reciprocal_approx_fast(out=rf, in_=s0f)
                rb = sp2.tile([128, NL], bf16, tag="rb", name="rb")
                nc.vector.tensor_copy(rb, rf)
                # tmp4[:, b] = R + 1/S1[b]; A = tmp4 * E in one wide mul
                tmp4 = sp2.tile([128, B, NL], bf16, tag="tmp4", name="tmp4")
                for b in range(B):
                    col = mt * 4 + b
                    nc.vector.tensor_scalar(
                        out=tmp4[:, b, :],
                        in0=rb,
                        scalar1=r1_sb[:, col : col + 1],
                        scalar2=None,
                        op0=OP.add,
                    )
                nc.vector.tensor_mul(e4, tmp4, e4)
                g = mt // 4
                ml = (mt % 4) * 128
                for b in range(B):
                    vt = vtp.tile([128, C], bf16, tag="vt", name="vt")
                    nc.sync.dma_start(out=vt, in_=v_out[g, b, ml : ml + 128, :])
                    for ch in range(2):
                        nc.tensor.matmul(
                            fusion_ps[b][ch],
                            vt[:, ch * 128 : (ch + 1) * 128],
                            e4[:, b, :],
                            start=(mt == 0),
                            stop=(mt == MT - 1),
                        )

            # ---- residual: fusion_pad = g_b * fusion + x ----------------
            for b in range(B):
                for ch in range(2):
                    nc.vector.scalar_tensor_tensor(
                        out=fp_sb[b][ch][:, 1:9, 2:66],
                        in0=fusion_ps[b][ch].rearrange("p (r w) -> p r w", w=Wd),
                        scalar=g_bcast[:, b : b + 1],
                        in1=x_sb[b][ch][:, 2 : 2 + NL].rearrange(
                            "p (r w) -> p r w", w=Wd
                        ),
                        op0=OP.mult,
                        op1=OP.add,
                    )

        # =================================================================
        # phase E: ConvTranspose2d -> 18-row output slab (host-stitched)
        # tap-outer loop so the 4 batches reuse each weight tile; the
        # (py,px) component is interleaved into a [128, 9, 128] stage so
        # the output DMA writes 512B-contiguous runs.
        # =================================================================
        with (
            tc.tile_pool(name="cps", bufs=1, space="PSUM") as cps,
            tc.tile_pool(name="osb", bufs=1) as osb,
        ):
            NOUT = 9 * Wd  # 576 spatial outputs per (b, py, px)
            for py in range(2):
                ost = [
                    osb.tile([128, 9, 2 * Wd], f32, tag=f"ost{b}", name=f"ost{b}")
                    for b in range(B)
                ]
                for px in range(2):
                    pss = [
                        cps.tile([128, NOUT], f32, tag=f"cps{b}", name=f"cps{b}")
                        for b in range(B)
                    ]
                    taps = [
                        (ky, kx, k)
                        for ky in (py, py + 2)
                        for kx in (px, px + 2)
                        for k in range(2)
                    ]
                    for ti, (ky, kx, k) in enumerate(taps):
                        ro = (py + ky) // 2 - py
                        ww = (px + kx) // 2 - 1
                        for b in range(B):
                            fp = fp_sb[b][k]
                            nc.tensor.matmul(
                                pss[b][:, 0:512],
                                wco_sb[ky][kx][k],
                                fp[:, ro : ro + 8, 2 + ww : 66 + ww],
                                start=(ti == 0),
                                stop=(ti == len(taps) - 1),
                            )
                            nc.tensor.matmul(
                                pss[b][:, 512:NOUT],
                                wco_sb[ky][kx][k],
                                fp[:, ro + 8, 2 + ww : 66 + ww],
                                start=(ti == 0),
                                stop=(ti == len(taps) - 1),
                            )
                    for b in range(B):
                        ov = ost[b].rearrange("p j (w q) -> p j w q", q=2)[
                            :, :, :, px
                        ]
                        psv = pss[b].rearrange("p (j w) -> p j w", w=Wd)
                        # bias on j=1..8 only: slab rows 0,1 (j=0) are
                        # completed by the neighbor's (biased) rows 16,17;
                        # global row 0 is patched on the host.
                        nc.scalar.activation(
                            out=ov[:, 1:9, :],
                            in_=psv[:, 1:9, :],
                            func=AF.Identity,
                            bias=bco_sb,
                        )
                        nc.scalar.activation(
                            out=ov[:, 0:1, :],
                            in_=psv[:, 0:1, :],
                            func=AF.Copy,
                        )
                for b in range(B):
                    nc.sync.dma_start(
                        out=out_p[b].rearrange("c (j t) w -> c j t w", t=2)[
                            :, :, 1 - py, :
                        ],
                        in_=ost[b],
                    )

    nc.finalize()
    return nc


# ---------------------------------------------------------------------------
# host side
# ---------------------------------------------------------------------------
def _host_prep(x, wq, bq, wv, bv, w_adj1, b_adj1, w_adj2, b_adj2, gamma, w_co, b_co):
    import ml_dtypes

    bf16 = ml_dtypes.bfloat16
    x = np.asarray(x, np.float32).reshape(B, C, HW)
    xpad = np.zeros((B, C, HW + 4), np.float32)
    xpad[:, :, 2 : 2 + HW] = x
    xpad = xpad.astype(bf16)

    wqT = np.ascontiguousarray(np.asarray(wq, np.float32).T).astype(bf16)
    wvT = np.ascontiguousarray(np.asarray(wv, np.float32).T).astype(bf16)

    # grouped conv -> block-diagonal [3, 256, 32]
    w1 = np.zeros((3, C, CR), np.float32)
    wa1 = np.asarray(w_adj1, np.float32)  # [32, 8, 3]
    for g in range(CR):
        w1[:, g * 8 : (g + 1) * 8, g] = wa1[g].T  # [8,3] -> [3,8]
    w1 = w1.astype(bf16)

    # conv2 with output channels permuted to [query(32) | key(32)]
    wa2 = np.asarray(w_adj2, np.float32)  # [64, 32, 3]
    perm = np.concatenate([np.arange(0, 64, 2), np.arange(1, 64, 2)])
    w2 = np.ascontiguousarray(wa2[perm].transpose(2, 1, 0)).astype(bf16)  # [3,32,64]
    b2p = np.ascontiguousarray(np.asarray(b_adj2, np.float32)[perm].reshape(2 * CR, 1))

    # convT weights: flip, swap I/O -> [ky, kx, c_in, c_out]
    wt = np.flip(np.asarray(w_co, np.float32), (2, 3)).transpose(1, 0, 2, 3)
    wco = np.ascontiguousarray(wt.transpose(2, 3, 1, 0)).astype(bf16)  # [4,4,256,128]

    bvb = np.ascontiguousarray(
        np.broadcast_to(np.asarray(bv, np.float32), (128, C)).astype(bf16)
    )
    bq_ = np.ascontiguousarray(np.asarray(bq, np.float32).reshape(C, 1))
    b1_ = np.ascontiguousarray(np.asarray(b_adj1, np.float32).reshape(CR, 1))
    bco_ = np.ascontiguousarray(np.asarray(b_co, np.float32).reshape(C // 2, 1))
    gm = np.ascontiguousarray(np.asarray(gamma, np.float32).reshape(1, 1))

    in_maps = []
    for i in range(NCORES):
        n0 = i * NL
        xsl = np.ascontiguousarray(xpad[:, :, n0 : n0 + XW])
        j = np.arange(XW)
        valid = ((n0 - 2 + j) >= 0) & ((n0 - 2 + j) < HW)
        mask = np.ascontiguousarray(
            np.broadcast_to(valid.astype(np.float32), (128, XW)).astype(bf16)
        )
        in_maps.append(
            dict(
                xs=xsl,
                wqT=wqT,
                wvT=wvT,
                w1=w1,
                w2=w2,
                wco=wco,
                bq=bq_,
                b1=b1_,
                b2p=b2p,
                bco=bco_,
                bvb=bvb,
                mask=mask,
                gammas=gm,
            )
        )
    return in_maps


def _stitch(outs):
    full = np.zeros((B, C // 2, 2 * H, 2 * Wd), np.float32)
    for i in range(NCORES):
        y0 = 16 * i - 1
        lo = max(0, y0)
        hi = min(2 * H, y0 + OUTROWS)
        full[:, :, lo:hi, :] += outs[i][:, :, lo - y0 : hi - y0, :]
    return full


def _get_nc():
    if "nc" not in _CACHE:
        _CACHE["nc"] = build_module()
    return _CACHE["nc"]


def run_spmd(in_maps, trace=False, **kw):
    from concourse.bass_utils import run_bass_kernel_spmd

    nc = _get_nc()
    return run_bass_kernel_spmd(
        nc, in_maps, core_ids=list(range(NCORES)), trace=trace, **kw
    )


def kernel(x, wq, bq, wv, bv, w_adj1, b_adj1, w_adj2, b_adj2, gamma, w_co, b_co):
    in_maps = _host_prep(
        x, wq, bq, wv, bv, w_adj1, b_adj1, w_adj2, b_adj2, gamma, w_co, b_co
    )
    res = run_spmd(in_maps)
    full = _stitch([r["out"] for r in res.results])
    # slab rows 0,1 carry no bias (the neighbor's rows complete them);
    # global row 0 has no neighbor, so add the bias here.
    full[:, :, 0, :] += np.asarray(b_co, np.float32)[None, :, None]
    return full.astype(np.float32)



# revision 8
# speedup vs baseline: 1.1350x; 1.1350x over previous
"""BidirectionalAttention Trainium2 Bass kernel — 8-core SPMD, v2.

Decomposition (same math as the verified baseline):
  q path : 1x1 conv (matmul) -> grouped conv1d k=3 -> conv1d k=3
  attn   : E = exp(q^T k); both softmaxes share one exp:
             attn_f + attn_b = E * (1/S0[n,m] + 1/S1[b,m])
             S0 = sum_b E  (batch softmax denom, axis=0)
             S1 = sum_n E  (row softmax denom, axis=1) -> two AllReduces
  fusion = value @ (attn_f+attn_b)^T scaled by gamma*mean(x_b), + x
  ConvTranspose2d(k=4,s=2,p=1) via the 4-subkernel parity decomposition,
  18-row output slabs with additive 2-row seams stitched on the host.

v2 performance changes vs the baseline:
  - K/Q/V in fp8e4 (K and V AllGathers halve; the attention branch output
    is scaled by gamma*mean(x) ~ 1e-3 so it tolerates fp8 easily).
    V is upcast to bf16 on the Scalar engine before the fusion matmuls.
  - E stays bf16 (DVE 2x perf mode requires 2-byte dtypes end-to-end).
  - One K AllGather for all 4 batches (was 4, each paying the ~15us ncfw
    floor).  CC-queue order: K-AG -> V-AG -> AR1 -> AR2, sized so each
    hides under local compute.
  - Phase C: one exp per m-tile, S1 via a single DVE tensor_reduce into a
    bf16 row (2x mode), S0 via an add tree split DVE/GpSimd by mt parity,
    1/S0 cached in bf16 for phase D (32 x 1KB/lane).
  - Phase D: A = (1/S0 + 1/S1) * E as ONE scalar_tensor_tensor per batch
    (replaces 4 tensor_scalars + a [128,2048] multiply), all operands bf16
    so DVE runs 2x.  Fusion matmuls then stream back-to-back to keep the
    PE warm (HAM throttling halved the baseline's matmul rate).
  - Coalesced DMAs: one const pack, one x pack, per-b V stages, one wco
    load, one output DMA per parity row.  Output returned in bf16.
"""

import numpy as np

B = 4
C = 256
H = 64
Wd = 64
HW = H * Wd            # 4096
CR = 32                # C // 8
NCORES = 8
NL = HW // NCORES      # 512 owned attention rows (n) per core
HL = H // NCORES       # 8 owned image rows per core
MT = HW // 128         # 32 m-tiles of 128
XW = NL + 4            # x slab width (n halo +-2 for the two k=3 convs)
Q2W = NL + 2           # q2 width (halo +-1 for conv2)
ROWW = 68              # fusion_pad row width: [0,1]=zero, 2..65 data, [66,67]=zero
OUTROWS = 2 * HL + 2   # 18 output rows per core (2-row overlaps, host-stitched)

# const-pack column offsets (bf16 elements)
OFF_WQ = 0             # [2, 256]
OFF_WV = 512           # [2, 256]
OFF_W1 = 1024          # [3, 2, 32]
OFF_MASK = 1216        # [516]
OFF_BVB = 1732         # [256]
OFF_W2 = 1988          # rows 0:32, [3, 64]
CPCOLS = 2180

_CACHE = {}


# ---------------------------------------------------------------------------
# device module
# ---------------------------------------------------------------------------
def build_module():
    from contextlib import ExitStack

    import concourse.bass as bass
    import concourse.mybir as mybir
    from concourse import bacc
    from concourse.tile import TileContext

    f32 = mybir.dt.float32
    bf16 = mybir.dt.bfloat16
    f8 = mybir.dt.float8e4
    AF = mybir.ActivationFunctionType
    OP = mybir.AluOpType
    AX = mybir.AxisListType

    nc = bacc.Bacc(num_devices=NCORES)
    RG = [list(range(NCORES))]

    # ---- parameters (per-core) -------------------------------------------
    cpack_p = nc.declare_dram_parameter("cpack", [128, CPCOLS], bf16, isOutput=False)
    fpack_p = nc.declare_dram_parameter("fpack", [128, 6], f32, isOutput=False)
    xpack_p = nc.declare_dram_parameter("xpack", [128, B, 2, XW], bf16, isOutput=False)
    wco_p = nc.declare_dram_parameter("wco", [32, 128, 128], bf16, isOutput=False)
    out_p = nc.declare_dram_parameter(
        "out", [B, C // 2, OUTROWS, 2 * Wd], bf16, isOutput=True
    )

    with TileContext(nc) as tc, ExitStack() as ctx:
        # ---- long-lived pools -------------------------------------------
        const = ctx.enter_context(tc.tile_pool(name="const", bufs=1))
        xpool = ctx.enter_context(tc.tile_pool(name="xp", bufs=1))
        qkv = ctx.enter_context(tc.tile_pool(name="qkv", bufs=1))
        fpool = ctx.enter_context(tc.tile_pool(name="fp", bufs=1))
        dram = ctx.enter_context(tc.tile_pool(name="dram", bufs=1, space="DRAM"))

        # ---- DRAM bounce buffers ----------------------------------------
        k_in = dram.tile([B, CR, NL], f8, tag="k_in", name="k_in")
        k_out = dram.tile(
            [NCORES, B, CR, NL], f8, tag="k_out", name="k_out"
        )
        v_in = dram.tile([B, NL, C], f8, tag="v_in", name="v_in")
        v_out = dram.tile(
            [NCORES, B, NL, C], f8, tag="v_out", name="v_out"
        )
        ar1_in = dram.tile([128, 64], bf16, tag="ar1_in", name="ar1_in")
        ar1_out = dram.tile(
            [128, 64], bf16, tag="ar1_out", name="ar1_out"
        )
        ar2_in = dram.tile([128, 72], bf16, tag="ar2_in", name="ar2_in")
        ar2_out = dram.tile(
            [128, 72], bf16, tag="ar2_out", name="ar2_out"
        )
        g_dram = dram.tile([1, B], f32, tag="g_dram", name="g_dram")

        # ---- persistent SBUF state --------------------------------------
        fpk = const.tile([128, 6], f32, tag="fpk", name="fpk")
        nc.sync.dma_start(out=fpk, in_=fpack_p[:, :])
        xt = xpool.tile([128, B, 2, XW], bf16, tag="xt", name="xt")
        nc.sync.dma_start(out=xt, in_=xpack_p[:, :, :, :])

        s1p = qkv.tile([128, 136], bf16, tag="s1p", name="s1p")
        Q_all = qkv.tile([128, NL], f8, tag="Q", name="Q")
        K_all = qkv.tile([128, HW], f8, tag="K", name="K")
        r1a = qkv.tile([128, 64], bf16, tag="r1a", name="r1a")  # 1/S1, mt<16
        r1b = qkv.tile([128, 64], bf16, tag="r1b", name="r1b")  # 1/S1, mt>=16
        g_bcast = qkv.tile([128, B], f32, tag="gbc", name="gbc")
        a1o = qkv.tile([128, 64], bf16, tag="a1o", name="a1o")
        a2o = qkv.tile([128, 72], bf16, tag="a2o", name="a2o")

        fp_sb = [
            [
                fpool.tile([128, 10, ROWW], bf16, tag=f"fpad{b}_{ch}", name=f"fpad{b}_{ch}")
                for ch in range(2)
            ]
            for b in range(B)
        ]

        def bq_v(k):
            return fpk[:, k : k + 1]

        b1_v = fpk[0:CR, 2:3]
        b2q_v = fpk[0:CR, 3:4]
        b2k_v = fpk[CR : 2 * CR, 3:4]
        bco_v = fpk[:, 4:5]
        gm_v = fpk[0:1, 5:6]

        # =================================================================
        # phases A (q path) + B (value) under the scoped const pack
        # =================================================================
        with (
            tc.tile_pool(name="cpA", bufs=1) as cpA,
            tc.tile_pool(name="qtmp", bufs=2) as qtmp,
            tc.tile_pool(name="qps", bufs=2, space="PSUM") as qps,
            tc.tile_pool(name="q2ps", bufs=1, space="PSUM") as q2ps,
            tc.tile_pool(name="q3ps", bufs=1, space="PSUM") as q3ps,
            tc.tile_pool(name="vps", bufs=1, space="PSUM") as vps,
            tc.tile_pool(name="vst", bufs=2) as vst,
        ):
            cp = cpA.tile([128, CPCOLS], bf16, tag="cp", name="cp")
            nc.sync.dma_start(out=cp, in_=cpack_p[:, :])

            def wq_v(k):
                return cp[:, OFF_WQ + k * 256 : OFF_WQ + (k + 1) * 256]

            def wv_v(k):
                return cp[:, OFF_WV + k * 256 : OFF_WV + (k + 1) * 256]

            def w1_v(t, k):
                o = OFF_W1 + (t * 2 + k) * CR
                return cp[:, o : o + CR]

            def w2_v(t):
                o = OFF_W2 + t * 64
                return cp[0:CR, o : o + 64]

            mask_v = cp[:, OFF_MASK : OFF_MASK + XW]
            bvb_v = cp[:, OFF_BVB : OFF_BVB + C]

            # x partial sums (for gamma*mean(x)) at s1p cols 128 + b*2 + k
            with nc.allow_low_precision("x mean partials tolerate bf16"):
                for b in range(B):
                    for k in range(2):
                        cc = 128 + b * 2 + k
                        nc.vector.tensor_reduce(
                            out=s1p[:, cc : cc + 1],
                            in_=xt[:, b, k, 2 : 2 + NL],
                            axis=AX.X,
                            op=OP.add,
                        )

            # ---- phase A: q path per batch ------------------------------
            for b in range(B):
                q1_sb = []
                for mtile in range(2):
                    ps = qps.tile([128, XW], f32, tag="q1ps", name="q1ps")
                    for k in range(2):
                        for lo, hi in ((0, 512), (512, XW)):
                            nc.tensor.matmul(
                                ps[:, lo:hi],
                                wq_v(k)[:, mtile * 128 : (mtile + 1) * 128],
                                xt[:, b, k, lo:hi],
                                start=(k == 0),
                                stop=(k == 1),
                            )
                    q1 = qtmp.tile([128, XW], bf16, tag=f"q1_{mtile}", name=f"q1_{mtile}")
                    nc.scalar.activation(
                        out=q1, in_=ps, func=AF.Identity, bias=bq_v(mtile)
                    )
                    nc.vector.tensor_mul(q1, q1, mask_v)
                    q1_sb.append(q1)

                ps2 = q2ps.tile([CR, Q2W], f32, tag="q2ps", name="q2ps")
                for t in range(3):
                    for k in range(2):
                        st = t == 0 and k == 0
                        sp = t == 2 and k == 1
                        for lo, hi in ((0, 512), (512, Q2W)):
                            nc.tensor.matmul(
                                ps2[:, lo:hi],
                                w1_v(t, k),
                                q1_sb[k][:, lo + t : hi + t],
                                start=st,
                                stop=sp,
                            )
                q2 = qtmp.tile([CR, Q2W], bf16, tag="q2", name="q2")
                nc.scalar.activation(out=q2, in_=ps2, func=AF.Identity, bias=b1_v)
                nc.vector.tensor_mul(q2, q2, mask_v[:CR, 1 : 1 + Q2W])

                ps3 = q3ps.tile([2 * CR, NL], f32, tag="q3ps", name="q3ps")
                for t in range(3):
                    nc.tensor.matmul(
                        ps3,
                        w2_v(t),
                        q2[:, t : t + NL],
                        start=(t == 0),
                        stop=(t == 2),
                    )
                q3 = qtmp.tile([2 * CR, NL], f8, tag="q3", name="q3")
                nc.scalar.activation(
                    out=q3, in_=ps3, func=AF.Identity, bias=fpk[0 : 2 * CR, 3:4]
                )
                nc.sync.dma_start(
                    out=Q_all[CR * b : CR * (b + 1), :], in_=q3[0:CR, :]
                )
                nc.sync.dma_start(out=k_in[b], in_=q3[CR : 2 * CR, :])

            # single K AllGather for all 4 batches
            nc.gpsimd.collective_compute(
                "AllGather",
                OP.bypass,
                replica_groups=RG,
                ins=[k_in[:, :, :]],
                outs=[k_out[:, :, :, :]],
            )

            # ---- phase B: value^T shards, fp8 ---------------------------
            for b in range(B):
                vstage = vst.tile([128, 4, C], f8, tag="vstage", name="vstage")
                for ms in range(4):
                    psv = vps.tile([128, C], f32, tag="vpsm", name="vpsm")
                    for k in range(2):
                        nc.tensor.matmul(
                            psv,
                            xt[:, b, k, 2 + ms * 128 : 2 + (ms + 1) * 128],
                            wv_v(k),
                            start=(k == 0),
                            stop=(k == 1),
                        )
                    nc.vector.tensor_add(vstage[:, ms, :], psv, bvb_v)
                nc.sync.dma_start(
                    out=v_in[b].rearrange("(ms p) c -> p ms c", p=128), in_=vstage
                )

            # assemble K_all from the gathered shards (per-b: the SBUF dst
            # must keep a single partition dim)
            for b in range(B):
                nc.sync.dma_start(
                    out=K_all[CR * b : CR * (b + 1), :].rearrange(
                        "c (g m) -> c g m", g=NCORES
                    ),
                    in_=k_out[:, b].rearrange("g c m -> c g m"),
                )

        nc.gpsimd.collective_compute(
            "AllGather",
            OP.bypass,
            replica_groups=RG,
            ins=[v_in[:, :, :]],
            outs=[v_out[:, :, :, :]],
        )

        # =================================================================
        # phases C (QK + exp + denominators) and D (scale + fusion matmul)
        # =================================================================
        with tc.tile_pool(name="work", bufs=1) as work:
            e_sb = [
                work.tile([128, B, NL], bf16, tag=f"e{mt}", name=f"e{mt}")
                for mt in range(MT)
            ]
            rb_sb = [
                work.tile([128, NL], bf16, tag=f"rb{mt}", name=f"rb{mt}")
                for mt in range(MT)
            ]

            with (
                tc.tile_pool(name="qk", bufs=2, space="PSUM") as qk,
                tc.tile_pool(name="sc", bufs=2) as sc,
            ):
                for mt in range(MT):
                    ps4 = qk.tile([128, B, NL], f32, tag="e4ps", name="e4ps")
                    for b in range(B):
                        nc.tensor.matmul(
                            ps4[:, b, :],
                            K_all[CR * b : CR * (b + 1), mt * 128 : (mt + 1) * 128],
                            Q_all[CR * b : CR * (b + 1), :],
                            start=True,
                            stop=True,
                            tile_position=(CR * b, 0),
                        )
                    e4 = e_sb[mt]
                    nc.scalar.activation(out=e4, in_=ps4, func=AF.Exp)
                    # S1 partials: one reduce over n (innermost), bf16 out (2x)
                    with nc.allow_low_precision("S1 softmax denom tolerates bf16"):
                        nc.vector.tensor_reduce(
                            out=s1p[:, 4 * mt : 4 * mt + 4],
                            in_=e4,
                            axis=AX.X,
                            op=OP.add,
                        )
                    # S0 = sum_b E: add tree, split DVE/GpSimd by parity
                    s0f = sc.tile([128, NL], f32, tag="s0f", name="s0f")
                    if mt % 2 == 0:
                        t2 = sc.tile([128, 2, NL], bf16, tag="t2", name="t2")
                        nc.vector.tensor_add(t2, e4[:, 0:2, :], e4[:, 2:4, :])
                        nc.vector.tensor_add(s0f, t2[:, 0, :], t2[:, 1, :])
                    else:
                        s01 = sc.tile([128, NL], bf16, tag="s01", name="s01")
                        s23 = sc.tile([128, NL], bf16, tag="s23", name="s23")
                        nc.gpsimd.tensor_add(s01, e4[:, 0, :], e4[:, 1, :])
                        nc.gpsimd.tensor_add(s23, e4[:, 2, :], e4[:, 3, :])
                        nc.gpsimd.tensor_add(s0f, s01, s23)
                    rf = sc.tile([128, NL], f32, tag="rf", name="rf")
                    nc.vector.reciprocal_approx_fast(out=rf, in_=s0f)
                    if mt % 2 == 0:
                        nc.scalar.copy(out=rb_sb[mt], in_=rf)
                    else:
                        nc.vector.tensor_copy(rb_sb[mt], rf)

                    if mt == MT // 2 - 1:
                        nc.sync.dma_start(out=ar1_in[:, :], in_=s1p[:, 0:64])
                        nc.gpsimd.collective_compute(
                            "AllReduce", OP.add, replica_groups=RG,
                            ins=[ar1_in[:, :]], outs=[ar1_out[:, :]],
                        )
                        nc.sync.dma_start(out=a1o, in_=ar1_out[:, :])
                        s1f = sc.tile([128, 64], f32, tag="s1f", name="s1f")
                        nc.vector.tensor_copy(s1f, a1o)
                        r1f = sc.tile([128, 64], f32, tag="r1f", name="r1f")
                        nc.vector.reciprocal_approx_fast(out=r1f, in_=s1f)
                        nc.vector.tensor_copy(r1a, r1f)

                # second AR half: S1 cols 64..128 plus the x sums
                nc.sync.dma_start(out=ar2_in[:, 0:64], in_=s1p[:, 64:128])
                nc.sync.dma_start(out=ar2_in[:, 64:72], in_=s1p[:, 128:136])
                nc.gpsimd.collective_compute(
                    "AllReduce", OP.add, replica_groups=RG,
                    ins=[ar2_in[:, :]], outs=[ar2_out[:, :]],
                )
                nc.sync.dma_start(out=a2o, in_=ar2_out[:, :])
                s2f = sc.tile([128, 64], f32, tag="s2f", name="s2f")
                nc.vector.tensor_copy(s2f, a2o[:, 0:64])
                r2f = sc.tile([128, 64], f32, tag="r2f", name="r2f")
                nc.vector.reciprocal_approx_fast(out=r2f, in_=s2f)
                nc.vector.tensor_copy(r1b, r2f)

                # g_bcast[p, b] = gamma * mean(x[b])
                xps = sc.tile([1, 8], f32, tag="xps", name="xps")
                xsf = sc.tile([128, 8], f32, tag="xsf", name="xsf")
                nc.vector.tensor_copy(xsf, a2o[:, 64:72])
                nc.gpsimd.tensor_reduce(out=xps, in_=xsf, axis=AX.C, op=OP.add)
                xv = xps.rearrange("p (b k) -> p b k", b=B)
                g0 = sc.tile([1, B], f32, tag="g0", name="g0")
                nc.vector.tensor_add(g0, xv[:, :, 0], xv[:, :, 1])
                nc.vector.tensor_scalar(
                    out=g0,
                    in0=g0,
                    scalar1=gm_v,
                    scalar2=float(1.0 / (C * HW)),
                    op0=OP.mult,
                    op1=OP.mult,
                )
                nc.sync.dma_start(out=g_dram[:, :], in_=g0)
                nc.sync.dma_start(
                    out=g_bcast,
                    in_=bass.AP(
                        tensor=g_dram.tensor,
                        offset=g_dram.offset,
                        ap=[[0, 128], [1, B]],
                    ),
                )

            for b in range(B):
                for ch in range(2):
                    nc.gpsimd.memset(fp_sb[b][ch], 0.0)

            # ---- phase D: A = E*(1/S0 + 1/S1) in place; fusion matmuls --
            with (
                tc.tile_pool(name="fus", bufs=1, space="PSUM") as fus,
                tc.tile_pool(name="vtp", bufs=4) as vtp,
            ):
                fusion_ps = [
                    [
                        fus.tile([128, NL], f32, tag=f"f{b}_{ch}", name=f"f{b}_{ch}")
                        for ch in range(2)
                    ]
                    for b in range(B)
                ]
                for mt in range(MT):
                    g = mt // 4
                    ml = (mt % 4) * 128
                    vt8 = vtp.tile([128, B, C], f8, tag="vt8", name="vt8")
                    nc.sync.dma_start(
                        out=vt8, in_=v_out[g, :, ml : ml + 128, :].rearrange("b p c -> p b c")
                    )
                    vtb = vtp.tile([128, B, C], bf16, tag="vtb", name="vtb")
                    nc.scalar.copy(out=vtb, in_=vt8)
                    e4 = e_sb[mt]
                    r1h = r1a if mt < 16 else r1b
                    cb = (4 * mt) % 64
                    for b in range(B):
                        nc.vector.scalar_tensor_tensor(
                            out=e4[:, b, :],
                            in0=rb_sb[mt],
                            scalar=r1h[:, cb + b : cb + b + 1],
                            in1=e4[:, b, :],
                            op0=OP.add,
                            op1=OP.mult,
                        )
                    for b in range(B):
                        for ch in range(2):
                            nc.tensor.matmul(
                                fusion_ps[b][ch],
                                vtb[:, b, ch * 128 : (ch + 1) * 128],
                                e4[:, b, :],
                                start=(mt == 0),
                                stop=(mt == MT - 1),
                            )

                # ---- residual: fusion_pad = g_b * fusion + x ------------
                for b in range(B):
                    for ch in range(2):
                        nc.vector.scalar_tensor_tensor(
                            out=fp_sb[b][ch][:, 1:9, 2:66],
                            in0=fusion_ps[b][ch].rearrange("p (r w) -> p r w", w=Wd),
                            scalar=g_bcast[:, b : b + 1],
                            in1=xt[:, b, ch, 2 : 2 + NL].rearrange(
                                "p (r w) -> p r w", w=Wd
                            ),
                            op0=OP.mult,
                            op1=OP.add,
                        )

        # =================================================================
        # phase E: ConvTranspose2d -> 18-row output slab (host-stitched)
        # =================================================================
        with (
            tc.tile_pool(name="wtp", bufs=1) as wtp,
            tc.tile_pool(name="ostp", bufs=2) as ostp,
            tc.tile_pool(name="cps", bufs=1, space="PSUM") as cps,
        ):
            wt = wtp.tile([128, 32, 128], bf16, tag="wt", name="wt")
            nc.sync.dma_start(out=wt, in_=wco_p.rearrange("t p co -> p t co"))

            def wco_v(ky, kx, k):
                return wt[:, ky * 8 + kx * 2 + k, :]

            NOUT = 9 * Wd  # 576 spatial outputs per (b, py, px)
            for py in range(2):
                ost = ostp.tile([128, B, 9, 2 * Wd], bf16, tag="ost", name="ost")
                for px in range(2):
                    pss = [
                        cps.tile([128, NOUT], f32, tag=f"cps{b}", name=f"cps{b}")
                        for b in range(B)
                    ]
                    taps = [
                        (ky, kx, k)
                        for ky in (py, py + 2)
                        for kx in (px, px + 2)
                        for k in range(2)
                    ]
                    for ti, (ky, kx, k) in enumerate(taps):
                        ro = (py + ky) // 2 - py
                        ww = (px + kx) // 2 - 1
                        for b in range(B):
                            fp = fp_sb[b][k]
                            nc.tensor.matmul(
                                pss[b][:, 0:512],
                                wco_v(ky, kx, k),
                                fp[:, ro : ro + 8, 2 + ww : 66 + ww],
                                start=(ti == 0),
                                stop=(ti == len(taps) - 1),
                            )
                            nc.tensor.matmul(
                                pss[b][:, 512:NOUT],
                                wco_v(ky, kx, k),
                                fp[:, ro + 8, 2 + ww : 66 + ww],
                                start=(ti == 0),
                                stop=(ti == len(taps) - 1),
                            )
                    for b in range(B):
                        ov = ost[:, b].rearrange("p j (w q) -> p j w q", q=2)[
                            :, :, :, px
                        ]
                        psv = pss[b].rearrange("p (j w) -> p j w", w=Wd)
                        # bias on j=1..8 only: slab rows 0,1 (j=0) are
                        # completed by the neighbor's (biased) rows 16,17;
                        # global row 0 is patched on the host.
                        nc.scalar.activation(
                            out=ov[:, 1:9, :],
                            in_=psv[:, 1:9, :],
                            func=AF.Identity,
                            bias=bco_v,
                        )
                        nc.scalar.activation(
                            out=ov[:, 0:1, :],
                            in_=psv[:, 0:1, :],
                            func=AF.Copy,
                        )
                for b in range(B):
                    nc.sync.dma_start(
                        out=out_p[b].rearrange("c (j t) w -> c j t w", t=2)[
                            :, :, 1 - py, :
                        ],
                        in_=ost[:, b],
                    )

    nc.finalize()
    return nc


# ---------------------------------------------------------------------------
# host side
# ---------------------------------------------------------------------------
def _host_prep(x, wq, bq, wv, bv, w_adj1, b_adj1, w_adj2, b_adj2, gamma, w_co, b_co):
    import ml_dtypes

    bf16 = ml_dtypes.bfloat16
    x = np.asarray(x, np.float32).reshape(B, C, HW)
    xpad = np.zeros((B, C, HW + 4), np.float32)
    xpad[:, :, 2 : 2 + HW] = x

    wqT = np.ascontiguousarray(np.asarray(wq, np.float32).T)  # [C, C]
    wvT = np.ascontiguousarray(np.asarray(wv, np.float32).T)

    # grouped conv -> block-diagonal [3, 256, 32]
    w1 = np.zeros((3, C, CR), np.float32)
    wa1 = np.asarray(w_adj1, np.float32)  # [32, 8, 3]
    for g in range(CR):
        w1[:, g * 8 : (g + 1) * 8, g] = wa1[g].T  # [8,3] -> [3,8]

    # conv2 with output channels permuted to [query(32) | key(32)]
    wa2 = np.asarray(w_adj2, np.float32)  # [64, 32, 3]
    perm = np.concatenate([np.arange(0, 64, 2), np.arange(1, 64, 2)])
    w2 = np.ascontiguousarray(wa2[perm].transpose(2, 1, 0))  # [3, 32, 64]
    b2p = np.asarray(b_adj2, np.float32)[perm]

    # convT weights: flip, swap I/O -> [ky, kx, c_in, c_out] -> [32,128,128]
    wt = np.flip(np.asarray(w_co, np.float32), (2, 3)).transpose(1, 0, 2, 3)
    wco = np.ascontiguousarray(
        wt.transpose(2, 3, 1, 0).reshape(4, 4, 2, 128, 128).reshape(32, 128, 128)
    ).astype(bf16)

    # const pack (mask differs per core; rest shared)
    cbase = np.zeros((128, CPCOLS), np.float32)
    for k in range(2):
        cbase[:, OFF_WQ + k * 256 : OFF_WQ + (k + 1) * 256] = wqT[
            k * 128 : (k + 1) * 128, :
        ]
        cbase[:, OFF_WV + k * 256 : OFF_WV + (k + 1) * 256] = wvT[
            k * 128 : (k + 1) * 128, :
        ]
    for t in range(3):
        for k in range(2):
            o = OFF_W1 + (t * 2 + k) * CR
            cbase[:, o : o + CR] = w1[t, k * 128 : (k + 1) * 128, :]
        cbase[0:CR, OFF_W2 + t * 64 : OFF_W2 + (t + 1) * 64] = w2[t]
    cbase[:, OFF_BVB : OFF_BVB + C] = np.asarray(bv, np.float32)[None, :]

    # f32 pack: bq k0/k1, b1, b2(perm), bco, gamma
    fpack = np.zeros((128, 6), np.float32)
    bqf = np.asarray(bq, np.float32)
    fpack[:, 0] = bqf[0:128]
    fpack[:, 1] = bqf[128:256]
    fpack[0:CR, 2] = np.asarray(b_adj1, np.float32)
    fpack[0 : 2 * CR, 3] = b2p
    fpack[:, 4] = np.asarray(b_co, np.float32)
    fpack[0, 5] = np.asarray(gamma, np.float32).reshape(-1)[0]
    fpack = np.ascontiguousarray(fpack)

    in_maps = []
    for i in range(NCORES):
        n0 = i * NL
        xsl = xpad[:, :, n0 : n0 + XW]  # [B, C, XW]
        xpk = np.ascontiguousarray(
            xsl.reshape(B, 2, 128, XW).transpose(2, 0, 1, 3).astype(bf16)
        )
        j = np.arange(XW)
        valid = ((n0 - 2 + j) >= 0) & ((n0 - 2 + j) < HW)
        cpk = cbase.copy()
        cpk[:, OFF_MASK : OFF_MASK + XW] = valid.astype(np.float32)[None, :]
        in_maps.append(
            dict(
                cpack=np.ascontiguousarray(cpk.astype(bf16)),
                fpack=fpack,
                xpack=xpk,
                wco=wco,
            )
        )
    return in_maps


def _stitch(outs):
    full = np.zeros((B, C // 2, 2 * H, 2 * Wd), np.float32)
    for i in range(NCORES):
        y0 = 16 * i - 1
        lo = max(0, y0)
        hi = min(2 * H, y0 + OUTROWS)
        full[:, :, lo:hi, :] += np.asarray(
            outs[i][:, :, lo - y0 : hi - y0, :], np.float32
        )
    return full


def _get_nc():
    if "nc" not in _CACHE:
        _CACHE["nc"] = build_module()
    return _CACHE["nc"]


def run_spmd(in_maps, trace=False, **kw):
    from concourse.bass_utils import run_bass_kernel_spmd

    nc = _get_nc()
    return run_bass_kernel_spmd(
        nc, in_maps, core_ids=list(range(NCORES)), trace=trace, **kw
    )


def kernel(x, wq, bq, wv, bv, w_adj1, b_adj1, w_adj2, b_adj2, gamma, w_co, b_co):
    in_maps = _host_prep(
        x, wq, bq, wv, bv, w_adj1, b_adj1, w_adj2, b_adj2, gamma, w_co, b_co
    )
    res = run_spmd(in_maps)
    full = _stitch([r["out"] for r in res.results])
    # slab rows 0,1 carry no bias (the neighbor's rows complete them);
    # global row 0 has no neighbor, so add the bias here.
    full[:, :, 0, :] += np.asarray(b_co, np.float32)[None, :, None]
    return full.astype(np.float32)


# revision 15
# speedup vs baseline: 1.1557x; 1.0182x over previous
"""BidirectionalAttention Trainium2 Bass kernel — 8-core SPMD, v2.

Decomposition (same math as the verified baseline):
  q path : 1x1 conv (matmul) -> grouped conv1d k=3 -> conv1d k=3
  attn   : E = exp(q^T k); both softmaxes share one exp:
             attn_f + attn_b = E * (1/S0[n,m] + 1/S1[b,m])
             S0 = sum_b E  (batch softmax denom, axis=0)
             S1 = sum_n E  (row softmax denom, axis=1) -> two AllReduces
  fusion = value @ (attn_f+attn_b)^T scaled by gamma*mean(x_b), + x
  ConvTranspose2d(k=4,s=2,p=1) via the 4-subkernel parity decomposition,
  18-row output slabs with additive 2-row seams stitched on the host.

v2 performance changes vs the baseline:
  - K/Q/V in fp8e4 (K and V AllGathers halve; the attention branch output
    is scaled by gamma*mean(x) ~ 1e-3 so it tolerates fp8 easily).
    V is upcast to bf16 on the Scalar engine before the fusion matmuls.
  - E stays bf16 (DVE 2x perf mode requires 2-byte dtypes end-to-end).
  - One K AllGather for all 4 batches (was 4, each paying the ~15us ncfw
    floor).  CC-queue order: K-AG -> V-AG -> AR1 -> AR2, sized so each
    hides under local compute.
  - Phase C: one exp per m-tile, S1 via a single DVE tensor_reduce into a
    bf16 row (2x mode), S0 via an add tree split DVE/GpSimd by mt parity,
    1/S0 cached in bf16 for phase D (32 x 1KB/lane).
  - Phase D: A = (1/S0 + 1/S1) * E as ONE scalar_tensor_tensor per batch
    (replaces 4 tensor_scalars + a [128,2048] multiply), all operands bf16
    so DVE runs 2x.  Fusion matmuls then stream back-to-back to keep the
    PE warm (HAM throttling halved the baseline's matmul rate).
  - Coalesced DMAs: one const pack, one x pack, per-b V stages, one wco
    load, one output DMA per parity row.  Output returned in bf16.
"""

import numpy as np

B = 4
C = 256
H = 64
Wd = 64
HW = H * Wd            # 4096
CR = 32                # C // 8
NCORES = 8
NL = HW // NCORES      # 512 owned attention rows (n) per core
HL = H // NCORES       # 8 owned image rows per core
MT = HW // 128         # 32 m-tiles of 128
XW = NL + 4            # x slab width (n halo +-2 for the two k=3 convs)
Q2W = NL + 2           # q2 width (halo +-1 for conv2)
ROWW = 68              # fusion_pad row width: [0,1]=zero, 2..65 data, [66,67]=zero
OUTROWS = 2 * HL + 2   # 18 output rows per core (2-row overlaps, host-stitched)

# const-pack column offsets (bf16 elements)
OFF_WQ = 0             # [2, 256]
OFF_WV = 512           # [2, 256]
OFF_W1 = 1024          # [3, 2, 32]
OFF_MASK = 1216        # [516]
OFF_BVB = 1732         # [256]
OFF_W2 = 1988          # rows 0:32, [3, 64]
CPCOLS = 2180

_CACHE = {}


# ---------------------------------------------------------------------------
# device module
# ---------------------------------------------------------------------------
def build_module():
    from contextlib import ExitStack

    import concourse.bass as bass
    import concourse.mybir as mybir
    from concourse import bacc
    from concourse.tile import TileContext

    f32 = mybir.dt.float32
    bf16 = mybir.dt.bfloat16
    f8 = mybir.dt.float8e4
    AF = mybir.ActivationFunctionType
    OP = mybir.AluOpType
    AX = mybir.AxisListType

    nc = bacc.Bacc(num_devices=NCORES)
    RG = [list(range(NCORES))]

    # ---- parameters (per-core) -------------------------------------------
    cpack_p = nc.declare_dram_parameter("cpack", [128, CPCOLS], bf16, isOutput=False)
    fpack_p = nc.declare_dram_parameter("fpack", [128, 6], f32, isOutput=False)
    xpack_p = nc.declare_dram_parameter("xpack", [128, B, 2, XW], bf16, isOutput=False)
    wco_p = nc.declare_dram_parameter("wco", [32, 128, 128], bf16, isOutput=False)
    out_p = nc.declare_dram_parameter(
        "out", [B, C // 2, OUTROWS, 2 * Wd], bf16, isOutput=True
    )

    with TileContext(nc) as tc, ExitStack() as ctx:
        # ---- long-lived pools -------------------------------------------
        const = ctx.enter_context(tc.tile_pool(name="const", bufs=1))
        xpool = ctx.enter_context(tc.tile_pool(name="xp", bufs=1))
        qkv = ctx.enter_context(tc.tile_pool(name="qkv", bufs=1))
        fpool = ctx.enter_context(tc.tile_pool(name="fp", bufs=1))
        dram = ctx.enter_context(tc.tile_pool(name="dram", bufs=1, space="DRAM"))

        # ---- DRAM bounce buffers ----------------------------------------
        k_in = dram.tile([B, CR, NL], f8, tag="k_in", name="k_in")
        k_out = dram.tile(
            [NCORES, B, CR, NL], f8, tag="k_out", name="k_out"
        )
        v_in = dram.tile([B, NL, C], f8, tag="v_in", name="v_in")
        v_out = dram.tile(
            [NCORES, B, NL, C], f8, tag="v_out", name="v_out"
        )
        ar1_in = dram.tile([128, 64], f32, tag="ar1_in", name="ar1_in")
        ar1_out = dram.tile(
            [128, 64], f32, tag="ar1_out", name="ar1_out"
        )
        ar2_in = dram.tile([128, 72], f32, tag="ar2_in", name="ar2_in")
        ar2_out = dram.tile(
            [128, 72], f32, tag="ar2_out", name="ar2_out"
        )
        g_dram = dram.tile([1, B], f32, tag="g_dram", name="g_dram")

        # ---- persistent SBUF state --------------------------------------
        fpk = const.tile([128, 6], f32, tag="fpk", name="fpk")
        nc.sync.dma_start(out=fpk, in_=fpack_p[:, :])
        xt = xpool.tile([128, B, 2, XW], bf16, tag="xt", name="xt")
        nc.sync.dma_start(out=xt, in_=xpack_p[:, :, :, :])

        s1p = qkv.tile([128, 136], f32, tag="s1p", name="s1p")
        Q_all = qkv.tile([128, NL], f8, tag="Q", name="Q")
        K_all = qkv.tile([128, HW], f8, tag="K", name="K")
        r1a = qkv.tile([128, 64], bf16, tag="r1a", name="r1a")  # 1/S1, mt<16
        r1b = qkv.tile([128, 64], bf16, tag="r1b", name="r1b")  # 1/S1, mt>=16
        g_bcast = qkv.tile([128, B], f32, tag="gbc", name="gbc")
        a1o = qkv.tile([128, 64], f32, tag="a1o", name="a1o")
        a2o = qkv.tile([128, 72], f32, tag="a2o", name="a2o")

        fp_sb = [
            [
                fpool.tile([128, 10, ROWW], bf16, tag=f"fpad{b}_{ch}", name=f"fpad{b}_{ch}")
                for ch in range(2)
            ]
            for b in range(B)
        ]

        def bq_v(k):
            return fpk[:, k : k + 1]

        b1_v = fpk[0:CR, 2:3]
        b2q_v = fpk[0:CR, 3:4]
        b2k_v = fpk[CR : 2 * CR, 3:4]
        bco_v = fpk[:, 4:5]
        gm_v = fpk[0:1, 5:6]

        # =================================================================
        # phases A (q path) + B (value) under the scoped const pack
        # =================================================================
        with (
            tc.tile_pool(name="cpA", bufs=1) as cpA,
            tc.tile_pool(name="qtmp", bufs=2) as qtmp,
            tc.tile_pool(name="qps", bufs=2, space="PSUM") as qps,
            tc.tile_pool(name="q2ps", bufs=1, space="PSUM") as q2ps,
            tc.tile_pool(name="q3ps", bufs=1, space="PSUM") as q3ps,
            tc.tile_pool(name="vps", bufs=1, space="PSUM") as vps,
            tc.tile_pool(name="vst", bufs=2) as vst,
        ):
            cp = cpA.tile([128, CPCOLS], bf16, tag="cp", name="cp")
            nc.sync.dma_start(out=cp, in_=cpack_p[:, :])

            def wq_v(k):
                return cp[:, OFF_WQ + k * 256 : OFF_WQ + (k + 1) * 256]

            def wv_v(k):
                return cp[:, OFF_WV + k * 256 : OFF_WV + (k + 1) * 256]

            def w1_v(t, k):
                o = OFF_W1 + (t * 2 + k) * CR
                return cp[:, o : o + CR]

            def w2_v(t):
                o = OFF_W2 + t * 64
                return cp[0:CR, o : o + 64]

            mask_v = cp[:, OFF_MASK : OFF_MASK + XW]
            bvb_v = cp[:, OFF_BVB : OFF_BVB + C]

            # x partial sums (for gamma*mean(x)) at s1p cols 128 + b*2 + k
            for b in range(B):
                for k in range(2):
                    cc = 128 + b * 2 + k
                    nc.vector.tensor_reduce(
                        out=s1p[:, cc : cc + 1],
                        in_=xt[:, b, k, 2 : 2 + NL],
                        axis=AX.X,
                        op=OP.add,
                    )

            # ---- phase A: q path per batch ------------------------------
            for b in range(B):
                q1_sb = []
                for mtile in range(2):
                    ps = qps.tile([128, XW], f32, tag="q1ps", name="q1ps")
                    for k in range(2):
                        for lo, hi in ((0, 512), (512, XW)):
                            nc.tensor.matmul(
                                ps[:, lo:hi],
                                wq_v(k)[:, mtile * 128 : (mtile + 1) * 128],
                                xt[:, b, k, lo:hi],
                                start=(k == 0),
                                stop=(k == 1),
                            )
                    q1 = qtmp.tile([128, XW], bf16, tag=f"q1_{mtile}", name=f"q1_{mtile}")
                    nc.scalar.activation(
                        out=q1, in_=ps, func=AF.Identity, bias=bq_v(mtile)
                    )
                    nc.vector.tensor_mul(q1, q1, mask_v)
                    q1_sb.append(q1)

                ps2 = q2ps.tile([CR, Q2W], f32, tag="q2ps", name="q2ps")
                for t in range(3):
                    for k in range(2):
                        st = t == 0 and k == 0
                        sp = t == 2 and k == 1
                        for lo, hi in ((0, 512), (512, Q2W)):
                            nc.tensor.matmul(
                                ps2[:, lo:hi],
                                w1_v(t, k),
                                q1_sb[k][:, lo + t : hi + t],
                                start=st,
                                stop=sp,
                            )
                q2 = qtmp.tile([CR, Q2W], bf16, tag="q2", name="q2")
                nc.scalar.activation(out=q2, in_=ps2, func=AF.Identity, bias=b1_v)
                nc.vector.tensor_mul(q2, q2, mask_v[:CR, 1 : 1 + Q2W])

                ps3 = q3ps.tile([2 * CR, NL], f32, tag="q3ps", name="q3ps")
                for t in range(3):
                    nc.tensor.matmul(
                        ps3,
                        w2_v(t),
                        q2[:, t : t + NL],
                        start=(t == 0),
                        stop=(t == 2),
                    )
                q3 = qtmp.tile([2 * CR, NL], f8, tag="q3", name="q3")
                nc.scalar.activation(
                    out=q3, in_=ps3, func=AF.Identity, bias=fpk[0 : 2 * CR, 3:4]
                )
                nc.sync.dma_start(
                    out=Q_all[CR * b : CR * (b + 1), :], in_=q3[0:CR, :]
                )
                nc.sync.dma_start(out=k_in[b], in_=q3[CR : 2 * CR, :])

            # single K AllGather for all 4 batches
            nc.gpsimd.collective_compute(
                "AllGather",
                OP.bypass,
                replica_groups=RG,
                ins=[k_in[:, :, :]],
                outs=[k_out[:, :, :, :]],
            )

            # ---- phase B: value^T shards, fp8 ---------------------------
            for b in range(B):
                vstage = vst.tile([128, 4, C], f8, tag="vstage", name="vstage")
                for ms in range(4):
                    psv = vps.tile([128, C], f32, tag="vpsm", name="vpsm")
                    for k in range(2):
                        nc.tensor.matmul(
                            psv,
                            xt[:, b, k, 2 + ms * 128 : 2 + (ms + 1) * 128],
                            wv_v(k),
                            start=(k == 0),
                            stop=(k == 1),
                        )
                    nc.vector.tensor_add(vstage[:, ms, :], psv, bvb_v)
                nc.sync.dma_start(
                    out=v_in[b].rearrange("(ms p) c -> p ms c", p=128), in_=vstage
                )

            # assemble K_all from the gathered shards (per-b: the SBUF dst
            # must keep a single partition dim)
            for b in range(B):
                nc.sync.dma_start(
                    out=K_all[CR * b : CR * (b + 1), :].rearrange(
                        "c (g m) -> c g m", g=NCORES
                    ),
                    in_=k_out[:, b].rearrange("g c m -> c g m"),
                )

        nc.gpsimd.collective_compute(
            "AllGather",
            OP.bypass,
            replica_groups=RG,
            ins=[v_in[:, :, :]],
            outs=[v_out[:, :, :, :]],
        )

        # =================================================================
        # phases C (QK + exp + denominators) and D (scale + fusion matmul)
        # =================================================================
        with tc.tile_pool(name="work", bufs=1) as work:
            e_sb = [
                work.tile([128, B, NL], bf16, tag=f"e{mt}", name=f"e{mt}")
                for mt in range(MT)
            ]
            rb_sb = [
                work.tile([128, NL], bf16, tag=f"rb{mt}", name=f"rb{mt}")
                for mt in range(MT)
            ]

            with (
                tc.tile_pool(name="qk", bufs=2, space="PSUM") as qk,
                tc.tile_pool(name="sc", bufs=2) as sc,
            ):
                for mt in range(MT):
                    ps4 = qk.tile([128, B, NL], f32, tag="e4ps", name="e4ps")
                    for b in range(B):
                        nc.tensor.matmul(
                            ps4[:, b, :],
                            K_all[CR * b : CR * (b + 1), mt * 128 : (mt + 1) * 128],
                            Q_all[CR * b : CR * (b + 1), :],
                            start=True,
                            stop=True,
                            tile_position=(CR * b, 0),
                        )
                    e4 = e_sb[mt]
                    # per-b exp with accum_out: S1 partials fall out on Scalar
                    for b in range(B):
                        col = 4 * mt + b
                        nc.scalar.activation(
                            out=e4[:, b, :],
                            in_=ps4[:, b, :],
                            func=AF.Exp,
                            accum_out=s1p[:, col : col + 1],
                        )
                    # S0 = sum_b E: add tree, mostly on GpSimd (DVE is the
                    # end-to-end critical queue)
                    s0f = sc.tile([128, NL], f32, tag="s0f", name="s0f")
                    if mt % 4 == 0:
                        t2 = sc.tile([128, 2, NL], bf16, tag="t2", name="t2")
                        nc.vector.tensor_add(t2, e4[:, 0:2, :], e4[:, 2:4, :])
                        nc.vector.tensor_add(s0f, t2[:, 0, :], t2[:, 1, :])
                    else:
                        s01 = sc.tile([128, NL], bf16, tag="s01", name="s01")
                        s23 = sc.tile([128, NL], bf16, tag="s23", name="s23")
                        nc.gpsimd.tensor_add(s01, e4[:, 0, :], e4[:, 1, :])
                        nc.gpsimd.tensor_add(s23, e4[:, 2, :], e4[:, 3, :])
                        nc.gpsimd.tensor_add(s0f, s01, s23)
                    rf = sc.tile([128, NL], f32, tag="rf", name="rf")
                    nc.vector.reciprocal_approx_fast(out=rf, in_=s0f)
                    if mt % 2 == 0:
                        nc.scalar.copy(out=rb_sb[mt], in_=rf)
                    else:
                        nc.vector.tensor_copy(rb_sb[mt], rf)

                    if mt == MT // 2 - 1:
                        nc.sync.dma_start(out=ar1_in[:, :], in_=s1p[:, 0:64])
                        nc.gpsimd.collective_compute(
                            "AllReduce", OP.add, replica_groups=RG,
                            ins=[ar1_in[:, :]], outs=[ar1_out[:, :]],
                        )
                        nc.sync.dma_start(out=a1o, in_=ar1_out[:, :])
                        r1f = sc.tile([128, 64], f32, tag="r1f", name="r1f")
                        nc.vector.reciprocal_approx_fast(out=r1f, in_=a1o)
                        nc.vector.tensor_copy(r1a, r1f)

                # second AR half: S1 cols 64..128 plus the x sums
                nc.sync.dma_start(out=ar2_in[:, 0:64], in_=s1p[:, 64:128])
                nc.sync.dma_start(out=ar2_in[:, 64:72], in_=s1p[:, 128:136])
                nc.gpsimd.collective_compute(
                    "AllReduce", OP.add, replica_groups=RG,
                    ins=[ar2_in[:, :]], outs=[ar2_out[:, :]],
                )
                nc.sync.dma_start(out=a2o, in_=ar2_out[:, :])
                r2f = sc.tile([128, 64], f32, tag="r2f", name="r2f")
                nc.vector.reciprocal_approx_fast(out=r2f, in_=a2o[:, 0:64])
                nc.vector.tensor_copy(r1b, r2f)

                # g_bcast[p, b] = gamma * mean(x[b])
                xps = sc.tile([1, 8], f32, tag="xps", name="xps")
                nc.gpsimd.tensor_reduce(
                    out=xps, in_=a2o[:, 64:72], axis=AX.C, op=OP.add
                )
                xv = xps.rearrange("p (b k) -> p b k", b=B)
                g0 = sc.tile([1, B], f32, tag="g0", name="g0")
                nc.vector.tensor_add(g0, xv[:, :, 0], xv[:, :, 1])
                nc.vector.tensor_scalar(
                    out=g0,
                    in0=g0,
                    scalar1=gm_v,
                    scalar2=float(1.0 / (C * HW)),
                    op0=OP.mult,
                    op1=OP.mult,
                )
                nc.sync.dma_start(out=g_dram[:, :], in_=g0)
                nc.sync.dma_start(
                    out=g_bcast,
                    in_=bass.AP(
                        tensor=g_dram.tensor,
                        offset=g_dram.offset,
                        ap=[[0, 128], [1, B]],
                    ),
                )

            for b in range(B):
                for ch in range(2):
                    nc.gpsimd.memset(fp_sb[b][ch], 0.0)

            # ---- phase D: A = E*(1/S0 + 1/S1) in place; fusion matmuls --
            with (
                tc.tile_pool(name="fus", bufs=1, space="PSUM") as fus,
                tc.tile_pool(name="vtp", bufs=4) as vtp,
            ):
                fusion_ps = [
                    [
                        fus.tile([128, NL], f32, tag=f"f{b}_{ch}", name=f"f{b}_{ch}")
                        for ch in range(2)
                    ]
                    for b in range(B)
                ]
                for mt in range(MT):
                    g = mt // 4
                    ml = (mt % 4) * 128
                    vt8 = vtp.tile([128, B, C], f8, tag="vt8", name="vt8")
                    nc.sync.dma_start(
                        out=vt8, in_=v_out[g, :, ml : ml + 128, :].rearrange("b p c -> p b c")
                    )
                    e4 = e_sb[mt]
                    r1h = r1a if mt < 16 else r1b
                    cb = (4 * mt) % 64
                    for b in range(B):
                        nc.vector.scalar_tensor_tensor(
                            out=e4[:, b, :],
                            in0=rb_sb[mt],
                            scalar=r1h[:, cb + b : cb + b + 1],
                            in1=e4[:, b, :],
                            op0=OP.add,
                            op1=OP.mult,
                        )
                    for b in range(B):
                        for ch in range(2):
                            nc.tensor.matmul(
                                fusion_ps[b][ch],
                                vt8[:, b, ch * 128 : (ch + 1) * 128],
                                e4[:, b, :],
                                start=(mt == 0),
                                stop=(mt == MT - 1),
                            )

                # ---- residual: fusion_pad = g_b * fusion + x ------------
                for b in range(B):
                    for ch in range(2):
                        nc.vector.scalar_tensor_tensor(
                            out=fp_sb[b][ch][:, 1:9, 2:66],
                            in0=fusion_ps[b][ch].rearrange("p (r w) -> p r w", w=Wd),
                            scalar=g_bcast[:, b : b + 1],
                            in1=xt[:, b, ch, 2 : 2 + NL].rearrange(
                                "p (r w) -> p r w", w=Wd
                            ),
                            op0=OP.mult,
                            op1=OP.add,
                        )

        # =================================================================
        # phase E: ConvTranspose2d -> 18-row output slab (host-stitched)
        # =================================================================
        with (
            tc.tile_pool(name="wtp", bufs=1) as wtp,
            tc.tile_pool(name="ostp", bufs=2) as ostp,
            tc.tile_pool(name="cps", bufs=1, space="PSUM") as cps,
        ):
            wt = wtp.tile([128, 32, 128], bf16, tag="wt", name="wt")
            nc.sync.dma_start(out=wt, in_=wco_p.rearrange("t p co -> p t co"))

            def wco_v(ky, kx, k):
                return wt[:, ky * 8 + kx * 2 + k, :]

            NOUT = 9 * Wd  # 576 spatial outputs per (b, py, px)
            for py in range(2):
                ost = ostp.tile([128, B, 9, 2 * Wd], bf16, tag="ost", name="ost")
                for px in range(2):
                    pss = [
                        cps.tile([128, NOUT], f32, tag=f"cps{b}", name=f"cps{b}")
                        for b in range(B)
                    ]
                    taps = [
                        (ky, kx, k)
                        for ky in (py, py + 2)
                        for kx in (px, px + 2)
                        for k in range(2)
                    ]
                    for ti, (ky, kx, k) in enumerate(taps):
                        ro = (py + ky) // 2 - py
                        ww = (px + kx) // 2 - 1
                        for b in range(B):
                            fp = fp_sb[b][k]
                            nc.tensor.matmul(
                                pss[b][:, 0:512],
                                wco_v(ky, kx, k),
                                fp[:, ro : ro + 8, 2 + ww : 66 + ww],
                                start=(ti == 0),
                                stop=(ti == len(taps) - 1),
                            )
                            nc.tensor.matmul(
                                pss[b][:, 512:NOUT],
                                wco_v(ky, kx, k),
                                fp[:, ro + 8, 2 + ww : 66 + ww],
                                start=(ti == 0),
                                stop=(ti == len(taps) - 1),
                            )
                    for b in range(B):
                        ov = ost[:, b].rearrange("p j (w q) -> p j w q", q=2)[
                            :, :, :, px
                        ]
                        psv = pss[b].rearrange("p (j w) -> p j w", w=Wd)
                        # bias on j=1..8 only: slab rows 0,1 (j=0) are
                        # completed by the neighbor's (biased) rows 16,17;
                        # global row 0 is patched on the host.
                        nc.scalar.activation(
                            out=ov[:, 1:9, :],
                            in_=psv[:, 1:9, :],
                            func=AF.Identity,
                            bias=bco_v,
                        )
                        nc.scalar.activation(
                            out=ov[:, 0:1, :],
                            in_=psv[:, 0:1, :],
                            func=AF.Copy,
                        )
                for b in range(B):
                    nc.sync.dma_start(
                        out=out_p[b].rearrange("c (j t) w -> c j t w", t=2)[
                            :, :, 1 - py, :
                        ],
                        in_=ost[:, b],
                    )

    nc.finalize()
    return nc


# ---------------------------------------------------------------------------
# host side
# ---------------------------------------------------------------------------
def _host_prep(x, wq, bq, wv, bv, w_adj1, b_adj1, w_adj2, b_adj2, gamma, w_co, b_co):
    import ml_dtypes

    bf16 = ml_dtypes.bfloat16
    x = np.asarray(x, np.float32).reshape(B, C, HW)
    xpad = np.zeros((B, C, HW + 4), np.float32)
    xpad[:, :, 2 : 2 + HW] = x

    wqT = np.ascontiguousarray(np.asarray(wq, np.float32).T)  # [C, C]
    wvT = np.ascontiguousarray(np.asarray(wv, np.float32).T)

    # grouped conv -> block-diagonal [3, 256, 32]
    w1 = np.zeros((3, C, CR), np.float32)
    wa1 = np.asarray(w_adj1, np.float32)  # [32, 8, 3]
    for g in range(CR):
        w1[:, g * 8 : (g + 1) * 8, g] = wa1[g].T  # [8,3] -> [3,8]

    # conv2 with output channels permuted to [query(32) | key(32)]
    wa2 = np.asarray(w_adj2, np.float32)  # [64, 32, 3]
    perm = np.concatenate([np.arange(0, 64, 2), np.arange(1, 64, 2)])
    w2 = np.ascontiguousarray(wa2[perm].transpose(2, 1, 0))  # [3, 32, 64]
    b2p = np.asarray(b_adj2, np.float32)[perm]

    # convT weights: flip, swap I/O -> [ky, kx, c_in, c_out] -> [32,128,128]
    wt = np.flip(np.asarray(w_co, np.float32), (2, 3)).transpose(1, 0, 2, 3)
    wco = np.ascontiguousarray(
        wt.transpose(2, 3, 1, 0).reshape(4, 4, 2, 128, 128).reshape(32, 128, 128)
    ).astype(bf16)

    # const pack (mask differs per core; rest shared)
    cbase = np.zeros((128, CPCOLS), np.float32)
    for k in range(2):
        cbase[:, OFF_WQ + k * 256 : OFF_WQ + (k + 1) * 256] = wqT[
            k * 128 : (k + 1) * 128, :
        ]
        cbase[:, OFF_WV + k * 256 : OFF_WV + (k + 1) * 256] = wvT[
            k * 128 : (k + 1) * 128, :
        ]
    for t in range(3):
        for k in range(2):
            o = OFF_W1 + (t * 2 + k) * CR
            cbase[:, o : o + CR] = w1[t, k * 128 : (k + 1) * 128, :]
        cbase[0:CR, OFF_W2 + t * 64 : OFF_W2 + (t + 1) * 64] = w2[t]
    cbase[:, OFF_BVB : OFF_BVB + C] = np.asarray(bv, np.float32)[None, :]

    # f32 pack: bq k0/k1, b1, b2(perm), bco, gamma
    fpack = np.zeros((128, 6), np.float32)
    bqf = np.asarray(bq, np.float32)
    fpack[:, 0] = bqf[0:128]
    fpack[:, 1] = bqf[128:256]
    fpack[0:CR, 2] = np.asarray(b_adj1, np.float32)
    fpack[0 : 2 * CR, 3] = b2p
    fpack[:, 4] = np.asarray(b_co, np.float32)
    fpack[0, 5] = np.asarray(gamma, np.float32).reshape(-1)[0]
    fpack = np.ascontiguousarray(fpack)

    in_maps = []
    for i in range(NCORES):
        n0 = i * NL
        xsl = xpad[:, :, n0 : n0 + XW]  # [B, C, XW]
        xpk = np.ascontiguousarray(
            xsl.reshape(B, 2, 128, XW).transpose(2, 0, 1, 3).astype(bf16)
        )
        j = np.arange(XW)
        valid = ((n0 - 2 + j) >= 0) & ((n0 - 2 + j) < HW)
        cpk = cbase.copy()
        cpk[:, OFF_MASK : OFF_MASK + XW] = valid.astype(np.float32)[None, :]
        in_maps.append(
            dict(
                cpack=np.ascontiguousarray(cpk.astype(bf16)),
                fpack=fpack,
                xpack=xpk,
                wco=wco,
            )
        )
    return in_maps


def _stitch(outs):
    full = np.zeros((B, C // 2, 2 * H, 2 * Wd), np.float32)
    for i in range(NCORES):
        y0 = 16 * i - 1
        lo = max(0, y0)
        hi = min(2 * H, y0 + OUTROWS)
        full[:, :, lo:hi, :] += np.asarray(
            outs[i][:, :, lo - y0 : hi - y0, :], np.float32
        )
    return full


def _get_nc():
    if "nc" not in _CACHE:
        _CACHE["nc"] = build_module()
    return _CACHE["nc"]


def run_spmd(in_maps, trace=False, **kw):
    from concourse.bass_utils import run_bass_kernel_spmd

    nc = _get_nc()
    return run_bass_kernel_spmd(
        nc, in_maps, core_ids=list(range(NCORES)), trace=trace, **kw
    )


def kernel(x, wq, bq, wv, bv, w_adj1, b_adj1, w_adj2, b_adj2, gamma, w_co, b_co):
    in_maps = _host_prep(
        x, wq, bq, wv, bv, w_adj1, b_adj1, w_adj2, b_adj2, gamma, w_co, b_co
    )
    res = run_spmd(in_maps)
    full = _stitch([r["out"] for r in res.results])
    # slab rows 0,1 carry no bias (the neighbor's rows complete them);
    # global row 0 has no neighbor, so add the bias here.
    full[:, :, 0, :] += np.asarray(b_co, np.float32)[None, :, None]
    return full.astype(np.float32)


# revision 21
# speedup vs baseline: 1.2202x; 1.0558x over previous
"""BidirectionalAttention Trainium2 Bass kernel — 8-core SPMD, v2.

Decomposition (same math as the verified baseline):
  q path : 1x1 conv (matmul) -> grouped conv1d k=3 -> conv1d k=3
  attn   : E = exp(q^T k); both softmaxes share one exp:
             attn_f + attn_b = E * (1/S0[n,m] + 1/S1[b,m])
             S0 = sum_b E  (batch softmax denom, axis=0)
             S1 = sum_n E  (row softmax denom, axis=1) -> two AllReduces
  fusion = value @ (attn_f+attn_b)^T scaled by gamma*mean(x_b), + x
  ConvTranspose2d(k=4,s=2,p=1) via the 4-subkernel parity decomposition,
  18-row output slabs with additive 2-row seams stitched on the host.

v2 performance changes vs the baseline:
  - K/Q/V in fp8e4 (K and V AllGathers halve; the attention branch output
    is scaled by gamma*mean(x) ~ 1e-3 so it tolerates fp8 easily).
    V is upcast to bf16 on the Scalar engine before the fusion matmuls.
  - E stays bf16 (DVE 2x perf mode requires 2-byte dtypes end-to-end).
  - One K AllGather for all 4 batches (was 4, each paying the ~15us ncfw
    floor).  CC-queue order: K-AG -> V-AG -> AR1 -> AR2, sized so each
    hides under local compute.
  - Phase C: one exp per m-tile, S1 via a single DVE tensor_reduce into a
    bf16 row (2x mode), S0 via an add tree split DVE/GpSimd by mt parity,
    1/S0 cached in bf16 for phase D (32 x 1KB/lane).
  - Phase D: A = (1/S0 + 1/S1) * E as ONE scalar_tensor_tensor per batch
    (replaces 4 tensor_scalars + a [128,2048] multiply), all operands bf16
    so DVE runs 2x.  Fusion matmuls then stream back-to-back to keep the
    PE warm (HAM throttling halved the baseline's matmul rate).
  - Coalesced DMAs: one const pack, one x pack, per-b V stages, one wco
    load, one output DMA per parity row.  Output returned in bf16.
"""

import numpy as np

B = 4
C = 256
H = 64
Wd = 64
HW = H * Wd            # 4096
CR = 32                # C // 8
NCORES = 8
NL = HW // NCORES      # 512 owned attention rows (n) per core
HL = H // NCORES       # 8 owned image rows per core
MT = HW // 128         # 32 m-tiles of 128
XW = NL + 4            # x slab width (n halo +-2 for the two k=3 convs)
Q2W = NL + 2           # q2 width (halo +-1 for conv2)
ROWW = 68              # fusion_pad row width: [0,1]=zero, 2..65 data, [66,67]=zero
OUTROWS = 2 * HL + 2   # 18 output rows per core (2-row overlaps, host-stitched)

# const-pack column offsets (bf16 elements)
OFF_WQ = 0             # [2, 256]
OFF_WV = 512           # [2, 256]
OFF_W1 = 1024          # [3, 2, 32]
OFF_MASK = 1216        # [516]
OFF_BVB = 1732         # [256]
OFF_W2 = 1988          # rows 0:32, [3, 64]
CPCOLS = 2180

_CACHE = {}


# ---------------------------------------------------------------------------
# device module
# ---------------------------------------------------------------------------
def build_module():
    from contextlib import ExitStack

    import concourse.bass as bass
    import concourse.mybir as mybir
    from concourse import bacc
    from concourse.tile import TileContext

    f32 = mybir.dt.float32
    bf16 = mybir.dt.bfloat16
    f8 = mybir.dt.float8e4
    AF = mybir.ActivationFunctionType
    OP = mybir.AluOpType
    AX = mybir.AxisListType

    nc = bacc.Bacc(num_devices=NCORES)
    RG = [list(range(NCORES))]

    # ---- parameters (per-core) -------------------------------------------
    cpack_p = nc.declare_dram_parameter("cpack", [128, CPCOLS], bf16, isOutput=False)
    fpack_p = nc.declare_dram_parameter("fpack", [128, 8], f32, isOutput=False)
    xpack_p = nc.declare_dram_parameter("xpack", [128, B, 2, XW], bf16, isOutput=False)
    wco_p = nc.declare_dram_parameter("wco", [32, 128, 128], bf16, isOutput=False)
    out_p = nc.declare_dram_parameter(
        "out", [B, C // 2, OUTROWS, 2 * Wd], bf16, isOutput=True
    )

    with TileContext(nc) as tc, ExitStack() as ctx:
        # ---- long-lived pools -------------------------------------------
        const = ctx.enter_context(tc.tile_pool(name="const", bufs=1))
        xpool = ctx.enter_context(tc.tile_pool(name="xp", bufs=1))
        qkv = ctx.enter_context(tc.tile_pool(name="qkv", bufs=1))
        fpool = ctx.enter_context(tc.tile_pool(name="fp", bufs=1))
        dram = ctx.enter_context(tc.tile_pool(name="dram", bufs=1, space="DRAM"))

        # ---- DRAM bounce buffers ----------------------------------------
        k_in = dram.tile([B, CR, NL], f8, tag="k_in", name="k_in")
        k_out = dram.tile(
            [NCORES, B, CR, NL], f8, tag="k_out", name="k_out"
        )
        v_in = dram.tile([B, NL, C], f8, tag="v_in", name="v_in")
        v_out = dram.tile(
            [NCORES, B, NL, C], f8, tag="v_out", name="v_out"
        )
        ar1_in = dram.tile([128, 64], f32, tag="ar1_in", name="ar1_in")
        ar1_out = dram.tile(
            [128, 64], f32, tag="ar1_out", name="ar1_out"
        )
        ar2_in = dram.tile([128, 72], f32, tag="ar2_in", name="ar2_in")
        ar2_out = dram.tile(
            [128, 72], f32, tag="ar2_out", name="ar2_out"
        )
        g_dram = dram.tile([1, B], f32, tag="g_dram", name="g_dram")

        # ---- persistent SBUF state --------------------------------------
        fpk = const.tile([128, 8], f32, tag="fpk", name="fpk")
        nc.sync.dma_start(out=fpk, in_=fpack_p[:, :])
        xt = xpool.tile([128, B, 2, XW], bf16, tag="xt", name="xt")
        nc.sync.dma_start(out=xt, in_=xpack_p[:, :, :, :])

        s1p = qkv.tile([128, 136], f32, tag="s1p", name="s1p")
        Q_all = qkv.tile([128, NL], f8, tag="Q", name="Q")
        K_all = qkv.tile([128, HW], f8, tag="K", name="K")
        r1a = qkv.tile([128, 64], bf16, tag="r1a", name="r1a")  # 1/S1, mt<16
        r1b = qkv.tile([128, 64], bf16, tag="r1b", name="r1b")  # 1/S1, mt>=16
        g_bcast = qkv.tile([128, B], f32, tag="gbc", name="gbc")
        a1o = qkv.tile([128, 64], f32, tag="a1o", name="a1o")
        a2o = qkv.tile([128, 72], f32, tag="a2o", name="a2o")

        fp_sb = [
            [
                fpool.tile([128, 10, ROWW], bf16, tag=f"fpad{b}_{ch}", name=f"fpad{b}_{ch}")
                for ch in range(2)
            ]
            for b in range(B)
        ]

        def bq_v(k):
            return fpk[:, k : k + 1]

        b1_v = fpk[0:CR, 2:3]
        b2q_v = fpk[0:CR, 3:4]
        b2k_v = fpk[CR : 2 * CR, 3:4]
        bco_v = fpk[:, 4:5]
        gm_v = fpk[0:1, 5:6]
        nege2_v = fpk[:, 6:7]  # -2.0 exp bias (fp8 range)

        # =================================================================
        # phases A (q path) + B (value) under the scoped const pack
        # =================================================================
        with (
            tc.tile_pool(name="cpA", bufs=1) as cpA,
            tc.tile_pool(name="qtmp", bufs=2) as qtmp,
            tc.tile_pool(name="qps", bufs=2, space="PSUM") as qps,
            tc.tile_pool(name="q2ps", bufs=1, space="PSUM") as q2ps,
            tc.tile_pool(name="q3ps", bufs=1, space="PSUM") as q3ps,
            tc.tile_pool(name="vps", bufs=1, space="PSUM") as vps,
            tc.tile_pool(name="vst", bufs=2) as vst,
        ):
            cp = cpA.tile([128, CPCOLS], bf16, tag="cp", name="cp")
            nc.sync.dma_start(out=cp, in_=cpack_p[:, :])

            def wq_v(k):
                return cp[:, OFF_WQ + k * 256 : OFF_WQ + (k + 1) * 256]

            def wv_v(k):
                return cp[:, OFF_WV + k * 256 : OFF_WV + (k + 1) * 256]

            def w1_v(t, k):
                o = OFF_W1 + (t * 2 + k) * CR
                return cp[:, o : o + CR]

            def w2_v(t):
                o = OFF_W2 + t * 64
                return cp[0:CR, o : o + 64]

            mask_v = cp[:, OFF_MASK : OFF_MASK + XW]
            bvb_v = cp[:, OFF_BVB : OFF_BVB + C]

            # x partial sums (for gamma*mean(x)) at s1p cols 128 + b*2 + k
            for b in range(B):
                for k in range(2):
                    cc = 128 + b * 2 + k
                    nc.vector.tensor_reduce(
                        out=s1p[:, cc : cc + 1],
                        in_=xt[:, b, k, 2 : 2 + NL],
                        axis=AX.X,
                        op=OP.add,
                    )

            # ---- phase A: q path per batch ------------------------------
            for b in range(B):
                q1_sb = []
                for mtile in range(2):
                    ps = qps.tile([128, XW], f32, tag="q1ps", name="q1ps")
                    for k in range(2):
                        for lo, hi in ((0, 512), (512, XW)):
                            nc.tensor.matmul(
                                ps[:, lo:hi],
                                wq_v(k)[:, mtile * 128 : (mtile + 1) * 128],
                                xt[:, b, k, lo:hi],
                                start=(k == 0),
                                stop=(k == 1),
                            )
                    q1 = qtmp.tile([128, XW], bf16, tag=f"q1_{mtile}", name=f"q1_{mtile}")
                    nc.scalar.activation(
                        out=q1, in_=ps, func=AF.Identity, bias=bq_v(mtile)
                    )
                    nc.vector.tensor_mul(q1, q1, mask_v)
                    q1_sb.append(q1)

                ps2 = q2ps.tile([CR, Q2W], f32, tag="q2ps", name="q2ps")
                for t in range(3):
                    for k in range(2):
                        st = t == 0 and k == 0
                        sp = t == 2 and k == 1
                        for lo, hi in ((0, 512), (512, Q2W)):
                            nc.tensor.matmul(
                                ps2[:, lo:hi],
                                w1_v(t, k),
                                q1_sb[k][:, lo + t : hi + t],
                                start=st,
                                stop=sp,
                            )
                q2 = qtmp.tile([CR, Q2W], bf16, tag="q2", name="q2")
                nc.scalar.activation(out=q2, in_=ps2, func=AF.Identity, bias=b1_v)
                nc.vector.tensor_mul(q2, q2, mask_v[:CR, 1 : 1 + Q2W])

                ps3 = q3ps.tile([2 * CR, NL], f32, tag="q3ps", name="q3ps")
                for t in range(3):
                    nc.tensor.matmul(
                        ps3,
                        w2_v(t),
                        q2[:, t : t + NL],
                        start=(t == 0),
                        stop=(t == 2),
                    )
                q3 = qtmp.tile([2 * CR, NL], f8, tag="q3", name="q3")
                nc.scalar.activation(
                    out=q3, in_=ps3, func=AF.Identity, bias=fpk[0 : 2 * CR, 3:4]
                )
                nc.sync.dma_start(
                    out=Q_all[CR * b : CR * (b + 1), :], in_=q3[0:CR, :]
                )
                nc.sync.dma_start(out=k_in[b], in_=q3[CR : 2 * CR, :])

            # single K AllGather for all 4 batches
            nc.gpsimd.collective_compute(
                "AllGather",
                OP.bypass,
                replica_groups=RG,
                ins=[k_in[:, :, :]],
                outs=[k_out[:, :, :, :]],
            )

            # ---- phase B: value^T shards, fp8 ---------------------------
            for b in range(B):
                vstage = vst.tile([128, 4, C], f8, tag="vstage", name="vstage")
                for ms in range(4):
                    psv = vps.tile([128, C], f32, tag="vpsm", name="vpsm")
                    for k in range(2):
                        nc.tensor.matmul(
                            psv,
                            xt[:, b, k, 2 + ms * 128 : 2 + (ms + 1) * 128],
                            wv_v(k),
                            start=(k == 0),
                            stop=(k == 1),
                        )
                    nc.vector.tensor_add(vstage[:, ms, :], psv, bvb_v)
                nc.sync.dma_start(
                    out=v_in[b].rearrange("(ms p) c -> p ms c", p=128), in_=vstage
                )

            # assemble K_all from the gathered shards (per-b: the SBUF dst
            # must keep a single partition dim)
            for b in range(B):
                nc.sync.dma_start(
                    out=K_all[CR * b : CR * (b + 1), :].rearrange(
                        "c (g m) -> c g m", g=NCORES
                    ),
                    in_=k_out[:, b].rearrange("g c m -> c g m"),
                )

        nc.gpsimd.collective_compute(
            "AllGather",
            OP.bypass,
            replica_groups=RG,
            ins=[v_in[:, :, :]],
            outs=[v_out[:, :, :, :]],
        )

        # =================================================================
        # phases C (QK + exp + denominators) and D (scale + fusion matmul)
        # =================================================================
        with tc.tile_pool(name="work", bufs=1) as work:
            # E in fp8e4 (exp bias -2 keeps E' <= ~130 < 448), stored as
            # m-tile PAIRS [128, 2, B, NL] for DoubleRow fusion matmuls
            e2 = [
                work.tile([128, 2, B, NL], f8, tag=f"e{t}", name=f"e{t}")
                for t in range(MT // 2)
            ]
            rb_sb = [
                work.tile([128, NL], bf16, tag=f"rb{mt}", name=f"rb{mt}")
                for mt in range(MT)
            ]

            with (
                tc.tile_pool(name="qk", bufs=2, space="PSUM") as qk,
                tc.tile_pool(name="sc", bufs=2) as sc,
            ):
                for mt in range(MT):
                    ps4 = qk.tile([128, B, NL], f32, tag="e4ps", name="e4ps")
                    for b in range(B):
                        nc.tensor.matmul(
                            ps4[:, b, :],
                            K_all[CR * b : CR * (b + 1), mt * 128 : (mt + 1) * 128],
                            Q_all[CR * b : CR * (b + 1), :],
                            start=True,
                            stop=True,
                            tile_position=(CR * b, 0),
                        )
                    ev = e2[mt // 2][:, mt % 2]  # [128, B, NL] view
                    # S1 partials: split between Scalar (per-b exp accum_out)
                    # and DVE (merged exp + 1x reduce) to balance the queues
                    if mt % 8 < 3:
                        for b in range(B):
                            col = 4 * mt + b
                            nc.scalar.activation(
                                out=ev[:, b, :],
                                in_=ps4[:, b, :],
                                func=AF.Exp,
                                bias=nege2_v,
                                accum_out=s1p[:, col : col + 1],
                            )
                    else:
                        nc.scalar.activation(out=ev, in_=ps4, func=AF.Exp, bias=nege2_v)
                        nc.vector.tensor_reduce(
                            out=s1p[:, 4 * mt : 4 * mt + 4],
                            in_=ev,
                            axis=AX.X,
                            op=OP.add,
                        )
                    # S0 = sum_b E: add tree, mostly on GpSimd
                    s0f = sc.tile([128, NL], f32, tag="s0f", name="s0f")
                    if mt % 4 == 0:
                        t2 = sc.tile([128, 2, NL], bf16, tag="t2", name="t2")
                        nc.vector.tensor_add(t2, ev[:, 0:2, :], ev[:, 2:4, :])
                        nc.vector.tensor_add(s0f, t2[:, 0, :], t2[:, 1, :])
                    else:
                        s01 = sc.tile([128, NL], bf16, tag="s01", name="s01")
                        s23 = sc.tile([128, NL], bf16, tag="s23", name="s23")
                        nc.gpsimd.tensor_add(s01, ev[:, 0, :], ev[:, 1, :])
                        nc.gpsimd.tensor_add(s23, ev[:, 2, :], ev[:, 3, :])
                        nc.gpsimd.tensor_add(s0f, s01, s23)
                    rf = sc.tile([128, NL], f32, tag="rf", name="rf")
                    nc.vector.reciprocal_approx_fast(out=rf, in_=s0f)
                    if mt % 2 == 0:
                        nc.scalar.copy(out=rb_sb[mt], in_=rf)
                    else:
                        nc.vector.tensor_copy(rb_sb[mt], rf)

                    if mt == MT // 2 - 1:
                        nc.sync.dma_start(out=ar1_in[:, :], in_=s1p[:, 0:64])
                        nc.gpsimd.collective_compute(
                            "AllReduce", OP.add, replica_groups=RG,
                            ins=[ar1_in[:, :]], outs=[ar1_out[:, :]],
                        )
                        nc.sync.dma_start(out=a1o, in_=ar1_out[:, :])
                        r1f = sc.tile([128, 64], f32, tag="r1f", name="r1f")
                        nc.vector.reciprocal_approx_fast(out=r1f, in_=a1o)
                        nc.vector.tensor_copy(r1a, r1f)

                # second AR half: S1 cols 64..128 plus the x sums
                nc.sync.dma_start(out=ar2_in[:, 0:64], in_=s1p[:, 64:128])
                nc.sync.dma_start(out=ar2_in[:, 64:72], in_=s1p[:, 128:136])
                nc.gpsimd.collective_compute(
                    "AllReduce", OP.add, replica_groups=RG,
                    ins=[ar2_in[:, :]], outs=[ar2_out[:, :]],
                )
                nc.sync.dma_start(out=a2o, in_=ar2_out[:, :])
                r2f = sc.tile([128, 64], f32, tag="r2f", name="r2f")
                nc.vector.reciprocal_approx_fast(out=r2f, in_=a2o[:, 0:64])
                nc.vector.tensor_copy(r1b, r2f)

                # g_bcast[p, b] = gamma * mean(x[b])
                xps = sc.tile([1, 8], f32, tag="xps", name="xps")
                nc.gpsimd.tensor_reduce(
                    out=xps, in_=a2o[:, 64:72], axis=AX.C, op=OP.add
                )
                xv = xps.rearrange("p (b k) -> p b k", b=B)
                g0 = sc.tile([1, B], f32, tag="g0", name="g0")
                nc.vector.tensor_add(g0, xv[:, :, 0], xv[:, :, 1])
                nc.vector.tensor_scalar(
                    out=g0,
                    in0=g0,
                    scalar1=gm_v,
                    scalar2=float(1.0 / (C * HW)),
                    op0=OP.mult,
                    op1=OP.mult,
                )
                nc.sync.dma_start(out=g_dram[:, :], in_=g0)
                nc.sync.dma_start(
                    out=g_bcast,
                    in_=bass.AP(
                        tensor=g_dram.tensor,
                        offset=g_dram.offset,
                        ap=[[0, 128], [1, B]],
                    ),
                )

            for b in range(B):
                for ch in range(2):
                    nc.gpsimd.memset(fp_sb[b][ch], 0.0)

            # ---- phase D: A = E*(1/S0 + 1/S1) in place; fusion matmuls --
            with (
                tc.tile_pool(name="fus", bufs=1, space="PSUM") as fus,
                tc.tile_pool(name="vtp", bufs=4) as vtp,
            ):
                fusion_ps = [
                    [
                        fus.tile([128, NL], f32, tag=f"f{b}_{ch}", name=f"f{b}_{ch}")
                        for ch in range(2)
                    ]
                    for b in range(B)
                ]
                NP = MT // 2
                for t in range(NP):
                    g = t // 2
                    ml = (t % 2) * 256
                    vt8 = vtp.tile([128, 2, B, C], f8, tag="vt8", name="vt8")
                    for b in range(B):
                        nc.sync.dma_start(
                            out=vt8[:, :, b, :],
                            in_=v_out[g, b, ml : ml + 256, :].rearrange(
                                "(two p) c -> p two c", p=128
                            ),
                        )
                    et = e2[t]
                    for par in range(2):
                        mt = 2 * t + par
                        r1h = r1a if mt < 16 else r1b
                        cb = (4 * mt) % 64
                        for b in range(B):
                            nc.vector.scalar_tensor_tensor(
                                out=et[:, par, b, :],
                                in0=rb_sb[mt],
                                scalar=r1h[:, cb + b : cb + b + 1],
                                in1=et[:, par, b, :],
                                op0=OP.add,
                                op1=OP.mult,
                            )
                    for b in range(B):
                        for ch in range(2):
                            nc.tensor.matmul(
                                fusion_ps[b][ch],
                                vt8[:, :, b, ch * 128 : (ch + 1) * 128],
                                et[:, :, b, :],
                                start=(t == 0),
                                stop=(t == NP - 1),
                                perf_mode=mybir.MatmulPerfMode.DoubleRow,
                            )

                # ---- residual: fusion_pad = g_b * fusion + x ------------
                for b in range(B):
                    for ch in range(2):
                        nc.vector.scalar_tensor_tensor(
                            out=fp_sb[b][ch][:, 1:9, 2:66],
                            in0=fusion_ps[b][ch].rearrange("p (r w) -> p r w", w=Wd),
                            scalar=g_bcast[:, b : b + 1],
                            in1=xt[:, b, ch, 2 : 2 + NL].rearrange(
                                "p (r w) -> p r w", w=Wd
                            ),
                            op0=OP.mult,
                            op1=OP.add,
                        )

        # =================================================================
        # phase E: ConvTranspose2d -> 18-row output slab (host-stitched)
        # =================================================================
        with (
            tc.tile_pool(name="wtp", bufs=1) as wtp,
            tc.tile_pool(name="ostp", bufs=2) as ostp,
            tc.tile_pool(name="cps", bufs=1, space="PSUM") as cps,
        ):
            wt = wtp.tile([128, 32, 128], bf16, tag="wt", name="wt")
            nc.sync.dma_start(out=wt, in_=wco_p.rearrange("t p co -> p t co"))

            def wco_v(ky, kx, k):
                return wt[:, ky * 8 + kx * 2 + k, :]

            NOUT = 9 * Wd  # 576 spatial outputs per (b, py, px)
            for py in range(2):
                ost = ostp.tile([128, B, 9, 2 * Wd], bf16, tag="ost", name="ost")
                for px in range(2):
                    pss = [
                        cps.tile([128, NOUT], f32, tag=f"cps{b}", name=f"cps{b}")
                        for b in range(B)
                    ]
                    taps = [
                        (ky, kx, k)
                        for ky in (py, py + 2)
                        for kx in (px, px + 2)
                        for k in range(2)
                    ]
                    for ti, (ky, kx, k) in enumerate(taps):
                        ro = (py + ky) // 2 - py
                        ww = (px + kx) // 2 - 1
                        for b in range(B):
                            fp = fp_sb[b][k]
                            nc.tensor.matmul(
                                pss[b][:, 0:512],
                                wco_v(ky, kx, k),
                                fp[:, ro : ro + 8, 2 + ww : 66 + ww],
                                start=(ti == 0),
                                stop=(ti == len(taps) - 1),
                            )
                            nc.tensor.matmul(
                                pss[b][:, 512:NOUT],
                                wco_v(ky, kx, k),
                                fp[:, ro + 8, 2 + ww : 66 + ww],
                                start=(ti == 0),
                                stop=(ti == len(taps) - 1),
                            )
                    for b in range(B):
                        ov = ost[:, b].rearrange("p j (w q) -> p j w q", q=2)[
                            :, :, :, px
                        ]
                        psv = pss[b].rearrange("p (j w) -> p j w", w=Wd)
                        # bias on j=1..8 only: slab rows 0,1 (j=0) are
                        # completed by the neighbor's (biased) rows 16,17;
                        # global row 0 is patched on the host.
                        nc.scalar.activation(
                            out=ov[:, 1:9, :],
                            in_=psv[:, 1:9, :],
                            func=AF.Identity,
                            bias=bco_v,
                        )
                        nc.scalar.activation(
                            out=ov[:, 0:1, :],
                            in_=psv[:, 0:1, :],
                            func=AF.Copy,
                        )
                for b in range(B):
                    nc.sync.dma_start(
                        out=out_p[b].rearrange("c (j t) w -> c j t w", t=2)[
                            :, :, 1 - py, :
                        ],
                        in_=ost[:, b],
                    )

    nc.finalize()
    return nc


# ---------------------------------------------------------------------------
# host side
# ---------------------------------------------------------------------------
def _host_prep(x, wq, bq, wv, bv, w_adj1, b_adj1, w_adj2, b_adj2, gamma, w_co, b_co):
    import ml_dtypes

    bf16 = ml_dtypes.bfloat16
    x = np.asarray(x, np.float32).reshape(B, C, HW)
    xpad = np.zeros((B, C, HW + 4), np.float32)
    xpad[:, :, 2 : 2 + HW] = x

    wqT = np.ascontiguousarray(np.asarray(wq, np.float32).T)  # [C, C]
    wvT = np.ascontiguousarray(np.asarray(wv, np.float32).T)

    # grouped conv -> block-diagonal [3, 256, 32]
    w1 = np.zeros((3, C, CR), np.float32)
    wa1 = np.asarray(w_adj1, np.float32)  # [32, 8, 3]
    for g in range(CR):
        w1[:, g * 8 : (g + 1) * 8, g] = wa1[g].T  # [8,3] -> [3,8]

    # conv2 with output channels permuted to [query(32) | key(32)]
    wa2 = np.asarray(w_adj2, np.float32)  # [64, 32, 3]
    perm = np.concatenate([np.arange(0, 64, 2), np.arange(1, 64, 2)])
    w2 = np.ascontiguousarray(wa2[perm].transpose(2, 1, 0))  # [3, 32, 64]
    b2p = np.asarray(b_adj2, np.float32)[perm]

    # convT weights: flip, swap I/O -> [ky, kx, c_in, c_out] -> [32,128,128]
    wt = np.flip(np.asarray(w_co, np.float32), (2, 3)).transpose(1, 0, 2, 3)
    wco = np.ascontiguousarray(
        wt.transpose(2, 3, 1, 0).reshape(4, 4, 2, 128, 128).reshape(32, 128, 128)
    ).astype(bf16)

    # const pack (mask differs per core; rest shared)
    cbase = np.zeros((128, CPCOLS), np.float32)
    for k in range(2):
        cbase[:, OFF_WQ + k * 256 : OFF_WQ + (k + 1) * 256] = wqT[
            k * 128 : (k + 1) * 128, :
        ]
        cbase[:, OFF_WV + k * 256 : OFF_WV + (k + 1) * 256] = wvT[
            k * 128 : (k + 1) * 128, :
        ]
    for t in range(3):
        for k in range(2):
            o = OFF_W1 + (t * 2 + k) * CR
            cbase[:, o : o + CR] = w1[t, k * 128 : (k + 1) * 128, :]
        cbase[0:CR, OFF_W2 + t * 64 : OFF_W2 + (t + 1) * 64] = w2[t]
    cbase[:, OFF_BVB : OFF_BVB + C] = np.asarray(bv, np.float32)[None, :]

    # f32 pack: bq k0/k1, b1, b2(perm), bco, gamma
    fpack = np.zeros((128, 8), np.float32)
    bqf = np.asarray(bq, np.float32)
    fpack[:, 0] = bqf[0:128]
    fpack[:, 1] = bqf[128:256]
    fpack[0:CR, 2] = np.asarray(b_adj1, np.float32)
    fpack[0 : 2 * CR, 3] = b2p
    fpack[:, 4] = np.asarray(b_co, np.float32)
    fpack[0, 5] = np.asarray(gamma, np.float32).reshape(-1)[0]
    fpack[:, 6] = -2.0
    fpack = np.ascontiguousarray(fpack)

    in_maps = []
    for i in range(NCORES):
        n0 = i * NL
        xsl = xpad[:, :, n0 : n0 + XW]  # [B, C, XW]
        xpk = np.ascontiguousarray(
            xsl.reshape(B, 2, 128, XW).transpose(2, 0, 1, 3).astype(bf16)
        )
        j = np.arange(XW)
        valid = ((n0 - 2 + j) >= 0) & ((n0 - 2 + j) < HW)
        cpk = cbase.copy()
        cpk[:, OFF_MASK : OFF_MASK + XW] = valid.astype(np.float32)[None, :]
        in_maps.append(
            dict(
                cpack=np.ascontiguousarray(cpk.astype(bf16)),
                fpack=fpack,
                xpack=xpk,
                wco=wco,
            )
        )
    return in_maps


def _stitch(outs):
    full = np.zeros((B, C // 2, 2 * H, 2 * Wd), np.float32)
    for i in range(NCORES):
        y0 = 16 * i - 1
        lo = max(0, y0)
        hi = min(2 * H, y0 + OUTROWS)
        full[:, :, lo:hi, :] += np.asarray(
            outs[i][:, :, lo - y0 : hi - y0, :], np.float32
        )
    return full


def _get_nc():
    if "nc" not in _CACHE:
        _CACHE["nc"] = build_module()
    return _CACHE["nc"]


def run_spmd(in_maps, trace=False, **kw):
    from concourse.bass_utils import run_bass_kernel_spmd

    nc = _get_nc()
    return run_bass_kernel_spmd(
        nc, in_maps, core_ids=list(range(NCORES)), trace=trace, **kw
    )


def kernel(x, wq, bq, wv, bv, w_adj1, b_adj1, w_adj2, b_adj2, gamma, w_co, b_co):
    in_maps = _host_prep(
        x, wq, bq, wv, bv, w_adj1, b_adj1, w_adj2, b_adj2, gamma, w_co, b_co
    )
    res = run_spmd(in_maps)
    full = _stitch([r["out"] for r in res.results])
    # slab rows 0,1 carry no bias (the neighbor's rows complete them);
    # global row 0 has no neighbor, so add the bias here.
    full[:, :, 0, :] += np.asarray(b_co, np.float32)[None, :, None]
    return full.astype(np.float32)


# revision 28
# speedup vs baseline: 1.2332x; 1.0107x over previous
"""BidirectionalAttention Trainium2 Bass kernel — 8-core SPMD, v2.

Decomposition (same math as the verified baseline):
  q path : 1x1 conv (matmul) -> grouped conv1d k=3 -> conv1d k=3
  attn   : E = exp(q^T k); both softmaxes share one exp:
             attn_f + attn_b = E * (1/S0[n,m] + 1/S1[b,m])
             S0 = sum_b E  (batch softmax denom, axis=0)
             S1 = sum_n E  (row softmax denom, axis=1) -> two AllReduces
  fusion = value @ (attn_f+attn_b)^T scaled by gamma*mean(x_b), + x
  ConvTranspose2d(k=4,s=2,p=1) via the 4-subkernel parity decomposition,
  18-row output slabs with additive 2-row seams stitched on the host.

v2 performance changes vs the baseline:
  - K/Q/V in fp8e4 (K and V AllGathers halve; the attention branch output
    is scaled by gamma*mean(x) ~ 1e-3 so it tolerates fp8 easily).
    V is upcast to bf16 on the Scalar engine before the fusion matmuls.
  - E stays bf16 (DVE 2x perf mode requires 2-byte dtypes end-to-end).
  - One K AllGather for all 4 batches (was 4, each paying the ~15us ncfw
    floor).  CC-queue order: K-AG -> V-AG -> AR1 -> AR2, sized so each
    hides under local compute.
  - Phase C: one exp per m-tile, S1 via a single DVE tensor_reduce into a
    bf16 row (2x mode), S0 via an add tree split DVE/GpSimd by mt parity,
    1/S0 cached in bf16 for phase D (32 x 1KB/lane).
  - Phase D: A = (1/S0 + 1/S1) * E as ONE scalar_tensor_tensor per batch
    (replaces 4 tensor_scalars + a [128,2048] multiply), all operands bf16
    so DVE runs 2x.  Fusion matmuls then stream back-to-back to keep the
    PE warm (HAM throttling halved the baseline's matmul rate).
  - Coalesced DMAs: one const pack, one x pack, per-b V stages, one wco
    load, one output DMA per parity row.  Output returned in bf16.
"""

import numpy as np

B = 4
C = 256
H = 64
Wd = 64
HW = H * Wd            # 4096
CR = 32                # C // 8
NCORES = 8
NL = HW // NCORES      # 512 owned attention rows (n) per core
HL = H // NCORES       # 8 owned image rows per core
MT = HW // 128         # 32 m-tiles of 128
XW = NL + 4            # x slab width (n halo +-2 for the two k=3 convs)
Q2W = NL + 2           # q2 width (halo +-1 for conv2)
ROWW = 68              # fusion_pad row width: [0,1]=zero, 2..65 data, [66,67]=zero
OUTROWS = 2 * HL + 2   # 18 output rows per core (2-row overlaps, host-stitched)

# const-pack column offsets (bf16 elements)
OFF_WQ = 0             # [2, 256]
OFF_WV = 512           # [2, 256]
OFF_W1 = 1024          # [3, 2, 32]
OFF_MASK = 1216        # [516]
OFF_BVB = 1732         # [256]
OFF_W2 = 1988          # rows 0:32, [3, 64]
CPCOLS = 2180

_CACHE = {}


# ---------------------------------------------------------------------------
# device module
# ---------------------------------------------------------------------------
def build_module():
    from contextlib import ExitStack

    import concourse.bass as bass
    import concourse.mybir as mybir
    from concourse import bacc
    from concourse.tile import TileContext

    f32 = mybir.dt.float32
    bf16 = mybir.dt.bfloat16
    f8 = mybir.dt.float8e4
    AF = mybir.ActivationFunctionType
    OP = mybir.AluOpType
    AX = mybir.AxisListType

    nc = bacc.Bacc(num_devices=NCORES)
    RG = [list(range(NCORES))]

    # ---- parameters (per-core) -------------------------------------------
    cpack_p = nc.declare_dram_parameter("cpack", [128, CPCOLS], bf16, isOutput=False)
    fpack_p = nc.declare_dram_parameter("fpack", [128, 8], f32, isOutput=False)
    xpack_p = nc.declare_dram_parameter("xpack", [128, B, 2, XW], bf16, isOutput=False)
    wco_p = nc.declare_dram_parameter("wco", [32, 128, 128], bf16, isOutput=False)
    out_p = nc.declare_dram_parameter(
        "out", [B, C // 2, OUTROWS, 2 * Wd], bf16, isOutput=True
    )

    with TileContext(nc) as tc, ExitStack() as ctx:
        # ---- long-lived pools -------------------------------------------
        const = ctx.enter_context(tc.tile_pool(name="const", bufs=1))
        xpool = ctx.enter_context(tc.tile_pool(name="xp", bufs=1))
        qkv = ctx.enter_context(tc.tile_pool(name="qkv", bufs=1))
        fpool = ctx.enter_context(tc.tile_pool(name="fp", bufs=1))
        dram = ctx.enter_context(tc.tile_pool(name="dram", bufs=1, space="DRAM"))

        # ---- DRAM bounce buffers ----------------------------------------
        k_in = dram.tile([B, CR, NL], f8, tag="k_in", name="k_in")
        k_out = dram.tile(
            [NCORES, B, CR, NL], f8, tag="k_out", name="k_out"
        )
        v_in = dram.tile([B, NL, C], f8, tag="v_in", name="v_in")
        v_out = dram.tile(
            [NCORES, B, NL, C], f8, tag="v_out", name="v_out"
        )
        ar1_in = dram.tile([128, 64], f32, tag="ar1_in", name="ar1_in")
        ar1_out = dram.tile(
            [128, 64], f32, tag="ar1_out", name="ar1_out"
        )
        ar2_in = dram.tile([128, 72], f32, tag="ar2_in", name="ar2_in")
        ar2_out = dram.tile(
            [128, 72], f32, tag="ar2_out", name="ar2_out"
        )
        g_dram = dram.tile([1, B], f32, tag="g_dram", name="g_dram")
        warm_in = dram.tile([1, 4], f32, tag="warm_in", name="warm_in")
        warm_out = dram.tile([NCORES, 4], f32, tag="warm_out", name="warm_out")

        # warm-up rendezvous: absorb the cross-core NEFF-start skew under
        # phase A instead of paying it at the first real collective
        nc.gpsimd.collective_compute(
            "AllGather", OP.bypass, replica_groups=RG,
            ins=[warm_in[:, :]], outs=[warm_out[:, :]],
        )

        # ---- persistent SBUF state --------------------------------------
        fpk = const.tile([128, 8], f32, tag="fpk", name="fpk")
        nc.sync.dma_start(out=fpk, in_=fpack_p[:, :])
        xt = xpool.tile([128, B, 2, XW], bf16, tag="xt", name="xt")
        nc.sync.dma_start(out=xt, in_=xpack_p[:, :, :, :])

        s1p = qkv.tile([128, 136], f32, tag="s1p", name="s1p")
        Q_all = qkv.tile([128, NL], f8, tag="Q", name="Q")
        K_all = qkv.tile([128, HW], f8, tag="K", name="K")
        r1a = qkv.tile([128, 64], bf16, tag="r1a", name="r1a")  # 1/S1, mt<16
        r1b = qkv.tile([128, 64], bf16, tag="r1b", name="r1b")  # 1/S1, mt>=16
        g_bcast = qkv.tile([128, B], f32, tag="gbc", name="gbc")
        a1o = qkv.tile([128, 64], f32, tag="a1o", name="a1o")
        a2o = qkv.tile([128, 72], f32, tag="a2o", name="a2o")

        wt = const.tile([128, 32, 128], bf16, tag="wt", name="wt")
        nc.sync.dma_start(out=wt, in_=wco_p.rearrange("t p co -> p t co"))

        def wco_v(ky, kx, k):
            return wt[:, ky * 8 + kx * 2 + k, :]

        wt8 = const.tile([128, 32, 128], f8, tag="wt8", name="wt8")
        nc.scalar.copy(out=wt8, in_=wt)

        def wco_pair(ky, kx):
            return wt8[:, ky * 8 + kx * 2 : ky * 8 + kx * 2 + 2, :]

        # x in ConvT layout (halo rows/cols zero) and the staged convT(x)+bias
        fpx = [
            [
                fpool.tile([128, 10, ROWW], bf16, tag=f"fpx{b}_{ch}", name=f"fpx{b}_{ch}")
                for ch in range(2)
            ]
            for b in range(B)
        ]
        stg = fpool.tile([128, 2, 2, B, 9, Wd], bf16, tag="stg", name="stg")
        for b in range(B):
            for ch in range(2):
                nc.gpsimd.memset(fpx[b][ch], 0.0)
                nc.scalar.copy(
                    out=fpx[b][ch][:, 1:9, 2:66],
                    in_=xt[:, b, ch, 2 : 2 + NL].rearrange("p (r w) -> p r w", w=Wd),
                )

        def bq_v(k):
            return fpk[:, k : k + 1]

        b1_v = fpk[0:CR, 2:3]
        b2q_v = fpk[0:CR, 3:4]
        b2k_v = fpk[CR : 2 * CR, 3:4]
        bco_v = fpk[:, 4:5]
        gm_v = fpk[0:1, 5:6]
        nege2_v = fpk[:, 6:7]  # -2.0 exp bias (fp8 range)

        # =================================================================
        # phases A (q path) + B (value) under the scoped const pack
        # =================================================================
        with (
            tc.tile_pool(name="cpA", bufs=1) as cpA,
            tc.tile_pool(name="qtmp", bufs=2) as qtmp,
            tc.tile_pool(name="qps", bufs=2, space="PSUM") as qps,
            tc.tile_pool(name="q2ps", bufs=1, space="PSUM") as q2ps,
            tc.tile_pool(name="q3ps", bufs=1, space="PSUM") as q3ps,
            tc.tile_pool(name="vps", bufs=1, space="PSUM") as vps,
            tc.tile_pool(name="vst", bufs=2) as vst,
        ):
            cp = cpA.tile([128, CPCOLS], bf16, tag="cp", name="cp")
            nc.sync.dma_start(out=cp, in_=cpack_p[:, :])

            def wq_v(k):
                return cp[:, OFF_WQ + k * 256 : OFF_WQ + (k + 1) * 256]

            def wv_v(k):
                return cp[:, OFF_WV + k * 256 : OFF_WV + (k + 1) * 256]

            def w1_v(t, k):
                o = OFF_W1 + (t * 2 + k) * CR
                return cp[:, o : o + CR]

            def w2_v(t):
                o = OFF_W2 + t * 64
                return cp[0:CR, o : o + 64]

            mask_v = cp[:, OFF_MASK : OFF_MASK + XW]
            bvb_v = cp[:, OFF_BVB : OFF_BVB + C]

            # x partial sums (for gamma*mean(x)) at s1p cols 128 + b*2 + k
            for b in range(B):
                for k in range(2):
                    cc = 128 + b * 2 + k
                    nc.vector.tensor_reduce(
                        out=s1p[:, cc : cc + 1],
                        in_=xt[:, b, k, 2 : 2 + NL],
                        axis=AX.X,
                        op=OP.add,
                    )

            # ---- phase A: q path per batch ------------------------------
            for b in range(B):
                q1_sb = []
                for mtile in range(2):
                    ps = qps.tile([128, XW], f32, tag="q1ps", name="q1ps")
                    for k in range(2):
                        for lo, hi in ((0, 512), (512, XW)):
                            nc.tensor.matmul(
                                ps[:, lo:hi],
                                wq_v(k)[:, mtile * 128 : (mtile + 1) * 128],
                                xt[:, b, k, lo:hi],
                                start=(k == 0),
                                stop=(k == 1),
                            )
                    q1 = qtmp.tile([128, XW], bf16, tag=f"q1_{mtile}", name=f"q1_{mtile}")
                    nc.scalar.activation(
                        out=q1, in_=ps, func=AF.Identity, bias=bq_v(mtile)
                    )
                    nc.vector.tensor_mul(q1, q1, mask_v)
                    q1_sb.append(q1)

                ps2 = q2ps.tile([CR, Q2W], f32, tag="q2ps", name="q2ps")
                for t in range(3):
                    for k in range(2):
                        st = t == 0 and k == 0
                        sp = t == 2 and k == 1
                        for lo, hi in ((0, 512), (512, Q2W)):
                            nc.tensor.matmul(
                                ps2[:, lo:hi],
                                w1_v(t, k),
                                q1_sb[k][:, lo + t : hi + t],
                                start=st,
                                stop=sp,
                            )
                q2 = qtmp.tile([CR, Q2W], bf16, tag="q2", name="q2")
                nc.scalar.activation(out=q2, in_=ps2, func=AF.Identity, bias=b1_v)
                nc.vector.tensor_mul(q2, q2, mask_v[:CR, 1 : 1 + Q2W])

                ps3 = q3ps.tile([2 * CR, NL], f32, tag="q3ps", name="q3ps")
                for t in range(3):
                    nc.tensor.matmul(
                        ps3,
                        w2_v(t),
                        q2[:, t : t + NL],
                        start=(t == 0),
                        stop=(t == 2),
                    )
                q3 = qtmp.tile([2 * CR, NL], f8, tag="q3", name="q3")
                nc.scalar.activation(
                    out=q3, in_=ps3, func=AF.Identity, bias=fpk[0 : 2 * CR, 3:4]
                )
                nc.sync.dma_start(
                    out=Q_all[CR * b : CR * (b + 1), :], in_=q3[0:CR, :]
                )
                nc.sync.dma_start(out=k_in[b], in_=q3[CR : 2 * CR, :])

            # single K AllGather for all 4 batches
            nc.gpsimd.collective_compute(
                "AllGather",
                OP.bypass,
                replica_groups=RG,
                ins=[k_in[:, :, :]],
                outs=[k_out[:, :, :, :]],
            )

            # ---- phase B: value^T shards, fp8 ---------------------------
            for b in range(B):
                vstage = vst.tile([128, 4, C], f8, tag="vstage", name="vstage")
                for ms in range(4):
                    psv = vps.tile([128, C], f32, tag="vpsm", name="vpsm")
                    for k in range(2):
                        nc.tensor.matmul(
                            psv,
                            xt[:, b, k, 2 + ms * 128 : 2 + (ms + 1) * 128],
                            wv_v(k),
                            start=(k == 0),
                            stop=(k == 1),
                        )
                    nc.vector.tensor_add(vstage[:, ms, :], psv, bvb_v)
                nc.sync.dma_start(
                    out=v_in[b].rearrange("(ms p) c -> p ms c", p=128), in_=vstage
                )

            # assemble K_all from the gathered shards (per-b: the SBUF dst
            # must keep a single partition dim)
            for b in range(B):
                nc.sync.dma_start(
                    out=K_all[CR * b : CR * (b + 1), :].rearrange(
                        "c (g m) -> c g m", g=NCORES
                    ),
                    in_=k_out[:, b].rearrange("g c m -> c g m"),
                )

        nc.gpsimd.collective_compute(
            "AllGather",
            OP.bypass,
            replica_groups=RG,
            ins=[v_in[:, :, :]],
            outs=[v_out[:, :, :, :]],
        )

        # =================================================================
        # conv-x: ConvTranspose of the residual x, staged to SBUF (+bias).
        # Runs in the collective dead-zone; keeps the PE warm before C.
        # =================================================================
        NOUT = 9 * Wd  # 576 spatial outputs per (b, py, px)
        with tc.tile_pool(name="cvx", bufs=1, space="PSUM") as cvx:
            for py in range(2):
                for px in range(2):
                    psx = [
                        cvx.tile([128, NOUT], f32, tag=f"cvx{b}", name=f"cvx{b}")
                        for b in range(B)
                    ]
                    taps = [
                        (ky, kx, k)
                        for ky in (py, py + 2)
                        for kx in (px, px + 2)
                        for k in range(2)
                    ]
                    for ti, (ky, kx, k) in enumerate(taps):
                        ro = (py + ky) // 2 - py
                        ww = (px + kx) // 2 - 1
                        for b in range(B):
                            fp = fpx[b][k]
                            nc.tensor.matmul(
                                psx[b][:, 0:512],
                                wco_v(ky, kx, k),
                                fp[:, ro : ro + 8, 2 + ww : 66 + ww],
                                start=(ti == 0),
                                stop=(ti == len(taps) - 1),
                            )
                            nc.tensor.matmul(
                                psx[b][:, 512:NOUT],
                                wco_v(ky, kx, k),
                                fp[:, ro + 8, 2 + ww : 66 + ww],
                                start=(ti == 0),
                                stop=(ti == len(taps) - 1),
                            )
                    for b in range(B):
                        sv = stg[:, py, px, b]
                        pv = psx[b].rearrange("p (j w) -> p j w", w=Wd)
                        nc.scalar.activation(
                            out=sv[:, 1:9, :], in_=pv[:, 1:9, :],
                            func=AF.Identity, bias=bco_v,
                        )
                        nc.scalar.activation(
                            out=sv[:, 0:1, :], in_=pv[:, 0:1, :], func=AF.Copy,
                        )

        # =================================================================
        # phases C (QK + exp + denominators) and D (scale + fusion matmul)
        # =================================================================
        with tc.tile_pool(name="work", bufs=1) as work:
            # E in fp8e4 (exp bias -2 keeps E' <= ~130 < 448), stored as
            # m-tile PAIRS [128, 2, B, NL] for DoubleRow fusion matmuls
            e2 = [
                work.tile([128, 2, B, NL], f8, tag=f"e{t}", name=f"e{t}")
                for t in range(MT // 2)
            ]
            rb_sb = [
                work.tile([128, NL], bf16, tag=f"rb{mt}", name=f"rb{mt}")
                for mt in range(MT)
            ]

            with (
                tc.tile_pool(name="qk", bufs=2, space="PSUM") as qk,
                tc.tile_pool(name="sc", bufs=2) as sc,
            ):
                for mt in range(MT):
                    ps4 = qk.tile([128, B, NL], f32, tag="e4ps", name="e4ps")
                    for b in range(B):
                        nc.tensor.matmul(
                            ps4[:, b, :],
                            K_all[CR * b : CR * (b + 1), mt * 128 : (mt + 1) * 128],
                            Q_all[CR * b : CR * (b + 1), :],
                            start=True,
                            stop=True,
                            tile_position=(CR * b, 0),
                        )
                    ev = e2[mt // 2][:, mt % 2]  # [128, B, NL] view
                    # S1 partials: split between Scalar (per-b exp accum_out)
                    # and DVE (merged exp + 1x reduce) to balance the queues
                    if mt % 8 < 3:
                        for b in range(B):
                            col = 4 * mt + b
                            nc.scalar.activation(
                                out=ev[:, b, :],
                                in_=ps4[:, b, :],
                                func=AF.Exp,
                                bias=nege2_v,
                                accum_out=s1p[:, col : col + 1],
                            )
                    else:
                        nc.scalar.activation(out=ev, in_=ps4, func=AF.Exp, bias=nege2_v)
                        nc.vector.tensor_reduce(
                            out=s1p[:, 4 * mt : 4 * mt + 4],
                            in_=ev,
                            axis=AX.X,
                            op=OP.add,
                        )
                    # S0 = sum_b E: add tree, mostly on GpSimd
                    s0f = sc.tile([128, NL], f32, tag="s0f", name="s0f")
                    if mt % 4 == 0:
                        t2 = sc.tile([128, 2, NL], bf16, tag="t2", name="t2")
                        nc.vector.tensor_add(t2, ev[:, 0:2, :], ev[:, 2:4, :])
                        nc.vector.tensor_add(s0f, t2[:, 0, :], t2[:, 1, :])
                    else:
                        s01 = sc.tile([128, NL], bf16, tag="s01", name="s01")
                        s23 = sc.tile([128, NL], bf16, tag="s23", name="s23")
                        nc.gpsimd.tensor_add(s01, ev[:, 0, :], ev[:, 1, :])
                        nc.gpsimd.tensor_add(s23, ev[:, 2, :], ev[:, 3, :])
                        nc.gpsimd.tensor_add(s0f, s01, s23)
                    rf = sc.tile([128, NL], f32, tag="rf", name="rf")
                    nc.vector.reciprocal_approx_fast(out=rf, in_=s0f)
                    if mt % 2 == 0:
                        nc.scalar.copy(out=rb_sb[mt], in_=rf)
                    else:
                        nc.vector.tensor_copy(rb_sb[mt], rf)

                    if mt == MT // 2 - 1:
                        nc.sync.dma_start(out=ar1_in[:, :], in_=s1p[:, 0:64])
                        nc.gpsimd.collective_compute(
                            "AllReduce", OP.add, replica_groups=RG,
                            ins=[ar1_in[:, :]], outs=[ar1_out[:, :]],
                        )
                        nc.sync.dma_start(out=a1o, in_=ar1_out[:, :])
                        r1f = sc.tile([128, 64], f32, tag="r1f", name="r1f")
                        nc.vector.reciprocal_approx_fast(out=r1f, in_=a1o)
                        nc.vector.tensor_copy(r1a, r1f)

                # second AR half: S1 cols 64..128 plus the x sums
                nc.sync.dma_start(out=ar2_in[:, 0:64], in_=s1p[:, 64:128])
                nc.sync.dma_start(out=ar2_in[:, 64:72], in_=s1p[:, 128:136])
                nc.gpsimd.collective_compute(
                    "AllReduce", OP.add, replica_groups=RG,
                    ins=[ar2_in[:, :]], outs=[ar2_out[:, :]],
                )
                nc.sync.dma_start(out=a2o, in_=ar2_out[:, :])
                r2f = sc.tile([128, 64], f32, tag="r2f", name="r2f")
                nc.vector.reciprocal_approx_fast(out=r2f, in_=a2o[:, 0:64])
                nc.vector.tensor_copy(r1b, r2f)

                # g_bcast[p, b] = gamma * mean(x[b])
                xps = sc.tile([1, 8], f32, tag="xps", name="xps")
                nc.gpsimd.tensor_reduce(
                    out=xps, in_=a2o[:, 64:72], axis=AX.C, op=OP.add
                )
                xv = xps.rearrange("p (b k) -> p b k", b=B)
                g0 = sc.tile([1, B], f32, tag="g0", name="g0")
                nc.vector.tensor_add(g0, xv[:, :, 0], xv[:, :, 1])
                nc.vector.tensor_scalar(
                    out=g0,
                    in0=g0,
                    scalar1=gm_v,
                    scalar2=float(4.0 / (C * HW)),
                    op0=OP.mult,
                    op1=OP.mult,
                )
                nc.sync.dma_start(out=g_dram[:, :], in_=g0)
                nc.sync.dma_start(
                    out=g_bcast,
                    in_=bass.AP(
                        tensor=g_dram.tensor,
                        offset=g_dram.offset,
                        ap=[[0, 128], [1, B]],
                    ),
                )

            # raw fusion in fp8 (|fusion| ~ 13 << 448); gamma*mean(x) is
            # applied in the phase-E epilogue (convT is linear, g is a
            # per-batch scalar)
            ff8 = [
                work.tile([128, 2, 10, ROWW], f8, tag=f"ff8{b}", name=f"ff8{b}")
                for b in range(B)
            ]
            for b in range(B):
                nc.gpsimd.memset(ff8[b], 0.0)

            # ---- phase D: A = E*(1/S0 + 1/S1) in place; fusion matmuls --
            with (
                tc.tile_pool(name="fus", bufs=1, space="PSUM") as fus,
                tc.tile_pool(name="vtp", bufs=4) as vtp,
            ):
                fusion_ps = [
                    [
                        fus.tile([128, NL], f32, tag=f"f{b}_{ch}", name=f"f{b}_{ch}")
                        for ch in range(2)
                    ]
                    for b in range(B)
                ]
                NP = MT // 2
                for t in range(NP):
                    g = t // 2
                    ml = (t % 2) * 256
                    vt8 = vtp.tile([128, 2, B, C], f8, tag="vt8", name="vt8")
                    for b in range(B):
                        nc.sync.dma_start(
                            out=vt8[:, :, b, :],
                            in_=v_out[g, b, ml : ml + 256, :].rearrange(
                                "(two p) c -> p two c", p=128
                            ),
                        )
                    et = e2[t]
                    for par in range(2):
                        mt = 2 * t + par
                        r1h = r1a if mt < 16 else r1b
                        cb = (4 * mt) % 64
                        for b in range(B):
                            nc.vector.scalar_tensor_tensor(
                                out=et[:, par, b, :],
                                in0=rb_sb[mt],
                                scalar=r1h[:, cb + b : cb + b + 1],
                                in1=et[:, par, b, :],
                                op0=OP.add,
                                op1=OP.mult,
                            )
                    for b in range(B):
                        for ch in range(2):
                            nc.tensor.matmul(
                                fusion_ps[b][ch],
                                vt8[:, :, b, ch * 128 : (ch + 1) * 128],
                                et[:, :, b, :],
                                start=(t == 0),
                                stop=(t == NP - 1),
                                perf_mode=mybir.MatmulPerfMode.DoubleRow,
                            )

                # ---- stage raw fusion to fp8 conv layout ----------------
                for b in range(B):
                    for ch in range(2):
                        # scale by 1/4: TRN fp8e4 max-normal is 240 and
                        # |fusion| reaches ~275; the epilogue g absorbs the 4x
                        nc.scalar.activation(
                            out=ff8[b][:, ch, 1:9, 2:66],
                            in_=fusion_ps[b][ch].rearrange("p (r w) -> p r w", w=Wd),
                            func=AF.Copy,
                            scale=0.25,
                        )

        # =================================================================
        # phase E: ConvTranspose2d of the fusion branch (fp8 DoubleRow over
        # the two c-chunks), epilogue out = g_b * conv_f + staged conv_x
        # =================================================================
        with (
            tc.tile_pool(name="ostp", bufs=2) as ostp,
            tc.tile_pool(name="cps", bufs=1, space="PSUM") as cps,
        ):
            for py in range(2):
                ost = ostp.tile([128, B, 9, 2 * Wd], bf16, tag="ost", name="ost")
                for px in range(2):
                    pss = [
                        cps.tile([128, NOUT], f32, tag=f"cps{b}", name=f"cps{b}")
                        for b in range(B)
                    ]
                    taps = [
                        (ky, kx)
                        for ky in (py, py + 2)
                        for kx in (px, px + 2)
                    ]
                    for ti, (ky, kx) in enumerate(taps):
                        ro = (py + ky) // 2 - py
                        ww = (px + kx) // 2 - 1
                        for b in range(B):
                            fp = ff8[b]
                            nc.tensor.matmul(
                                pss[b][:, 0:512],
                                wco_pair(ky, kx),
                                fp[:, :, ro : ro + 8, 2 + ww : 66 + ww],
                                start=(ti == 0),
                                stop=(ti == len(taps) - 1),
                                perf_mode=mybir.MatmulPerfMode.DoubleRow,
                            )
                            nc.tensor.matmul(
                                pss[b][:, 512:NOUT],
                                wco_pair(ky, kx),
                                fp[:, :, ro + 8, 2 + ww : 66 + ww],
                                start=(ti == 0),
                                stop=(ti == len(taps) - 1),
                                perf_mode=mybir.MatmulPerfMode.DoubleRow,
                            )
                    for b in range(B):
                        ov = ost[:, b].rearrange("p j (w q) -> p j w q", q=2)[
                            :, :, :, px
                        ]
                        psv = pss[b].rearrange("p (j w) -> p j w", w=Wd)
                        nc.vector.scalar_tensor_tensor(
                            out=ov,
                            in0=psv,
                            scalar=g_bcast[:, b : b + 1],
                            in1=stg[:, py, px, b],
                            op0=OP.mult,
                            op1=OP.add,
                        )
                for b in range(B):
                    nc.sync.dma_start(
                        out=out_p[b].rearrange("c (j t) w -> c j t w", t=2)[
                            :, :, 1 - py, :
                        ],
                        in_=ost[:, b],
                    )

    nc.finalize()
    return nc


# ---------------------------------------------------------------------------
# host side
# ---------------------------------------------------------------------------
def _host_prep(x, wq, bq, wv, bv, w_adj1, b_adj1, w_adj2, b_adj2, gamma, w_co, b_co):
    import ml_dtypes

    bf16 = ml_dtypes.bfloat16
    x = np.asarray(x, np.float32).reshape(B, C, HW)
    xpad = np.zeros((B, C, HW + 4), np.float32)
    xpad[:, :, 2 : 2 + HW] = x

    wqT = np.ascontiguousarray(np.asarray(wq, np.float32).T)  # [C, C]
    wvT = np.ascontiguousarray(np.asarray(wv, np.float32).T)

    # grouped conv -> block-diagonal [3, 256, 32]
    w1 = np.zeros((3, C, CR), np.float32)
    wa1 = np.asarray(w_adj1, np.float32)  # [32, 8, 3]
    for g in range(CR):
        w1[:, g * 8 : (g + 1) * 8, g] = wa1[g].T  # [8,3] -> [3,8]

    # conv2 with output channels permuted to [query(32) | key(32)]
    wa2 = np.asarray(w_adj2, np.float32)  # [64, 32, 3]
    perm = np.concatenate([np.arange(0, 64, 2), np.arange(1, 64, 2)])
    w2 = np.ascontiguousarray(wa2[perm].transpose(2, 1, 0))  # [3, 32, 64]
    b2p = np.asarray(b_adj2, np.float32)[perm]

    # convT weights: flip, swap I/O -> [ky, kx, c_in, c_out] -> [32,128,128]
    wt = np.flip(np.asarray(w_co, np.float32), (2, 3)).transpose(1, 0, 2, 3)
    wco = np.ascontiguousarray(
        wt.transpose(2, 3, 1, 0).reshape(4, 4, 2, 128, 128).reshape(32, 128, 128)
    ).astype(bf16)

    # const pack (mask differs per core; rest shared)
    cbase = np.zeros((128, CPCOLS), np.float32)
    for k in range(2):
        cbase[:, OFF_WQ + k * 256 : OFF_WQ + (k + 1) * 256] = wqT[
            k * 128 : (k + 1) * 128, :
        ]
        cbase[:, OFF_WV + k * 256 : OFF_WV + (k + 1) * 256] = wvT[
            k * 128 : (k + 1) * 128, :
        ]
    for t in range(3):
        for k in range(2):
            o = OFF_W1 + (t * 2 + k) * CR
            cbase[:, o : o + CR] = w1[t, k * 128 : (k + 1) * 128, :]
        cbase[0:CR, OFF_W2 + t * 64 : OFF_W2 + (t + 1) * 64] = w2[t]
    cbase[:, OFF_BVB : OFF_BVB + C] = np.asarray(bv, np.float32)[None, :]

    # f32 pack: bq k0/k1, b1, b2(perm), bco, gamma
    fpack = np.zeros((128, 8), np.float32)
    bqf = np.asarray(bq, np.float32)
    fpack[:, 0] = bqf[0:128]
    fpack[:, 1] = bqf[128:256]
    fpack[0:CR, 2] = np.asarray(b_adj1, np.float32)
    fpack[0 : 2 * CR, 3] = b2p
    fpack[:, 4] = np.asarray(b_co, np.float32)
    fpack[0, 5] = np.asarray(gamma, np.float32).reshape(-1)[0]
    fpack[:, 6] = -2.0
    fpack = np.ascontiguousarray(fpack)

    in_maps = []
    for i in range(NCORES):
        n0 = i * NL
        xsl = xpad[:, :, n0 : n0 + XW]  # [B, C, XW]
        xpk = np.ascontiguousarray(
            xsl.reshape(B, 2, 128, XW).transpose(2, 0, 1, 3).astype(bf16)
        )
        j = np.arange(XW)
        valid = ((n0 - 2 + j) >= 0) & ((n0 - 2 + j) < HW)
        cpk = cbase.copy()
        cpk[:, OFF_MASK : OFF_MASK + XW] = valid.astype(np.float32)[None, :]
        in_maps.append(
            dict(
                cpack=np.ascontiguousarray(cpk.astype(bf16)),
                fpack=fpack,
                xpack=xpk,
                wco=wco,
            )
        )
    return in_maps


def _stitch(outs):
    full = np.zeros((B, C // 2, 2 * H, 2 * Wd), np.float32)
    for i in range(NCORES):
        y0 = 16 * i - 1
        lo = max(0, y0)
        hi = min(2 * H, y0 + OUTROWS)
        full[:, :, lo:hi, :] += np.asarray(
            outs[i][:, :, lo - y0 : hi - y0, :], np.float32
        )
    return full


def _get_nc():
    if "nc" not in _CACHE:
        _CACHE["nc"] = build_module()
    return _CACHE["nc"]


def run_spmd(in_maps, trace=False, **kw):
    from concourse.bass_utils import run_bass_kernel_spmd

    nc = _get_nc()
    return run_bass_kernel_spmd(
        nc, in_maps, core_ids=list(range(NCORES)), trace=trace, **kw
    )


def kernel(x, wq, bq, wv, bv, w_adj1, b_adj1, w_adj2, b_adj2, gamma, w_co, b_co):
    in_maps = _host_prep(
        x, wq, bq, wv, bv, w_adj1, b_adj1, w_adj2, b_adj2, gamma, w_co, b_co
    )
    res = run_spmd(in_maps)
    full = _stitch([r["out"] for r in res.results])
    # slab rows 0,1 carry no bias (the neighbor's rows complete them);
    # global row 0 has no neighbor, so add the bias here.
    full[:, :, 0, :] += np.asarray(b_co, np.float32)[None, :, None]
    return full.astype(np.float32)
